# Initial kernel scaffold
#
"""Exphydro (snow + 2-bucket soil + gamma-UH routing) Trainium2 Bass kernel.

Contract: kernel(x_phy [1095,15000,3] f32, params [15000,16] f32) -> [1095,15000] f32.
Shards the grid dim across 8 NeuronCores (1875 -> padded 1920 per core).

Per-core pipeline (single NEFF, fully unrolled):
  phase A  per time-block: forcing streams (rain/snow on DVE, freeze-pot
           prep on Pool, melt-pot / ln / exp / scaled-pet on ScalarE) in
           [g-partition, t-free] layout with per-partition param scalars.
  phase B  sequential scan over time: 23 DVE ops (incl. 2 custom fused DVE
           ops) + 6 GpSimd ops per step on [128 part, 15 chunk] tiles with
           strided stream access; soil bucket tracked in V=vad/vm units.
  phase C  15-tap x 2 gamma-UH causal conv as diagonal matmuls on TensorE
           accumulating in PSUM, interleaved under the scan (weights
           pre-scaled by vm; lgamma cancels in the softmax normalization).
"""
import numpy as np

T = 1095
TB = 128
NB = 9
TPAD = TB * NB           # 1152
G = 15000
NCORES = 8
GC = 1920                # padded grid per core
NCH = 15                 # chunks of 128 per core
L = 15                   # UH length
NZ = 1e-5

_CACHE = {}


def _build_program():
    import concourse.bass as bass
    import concourse.mybir as mybir
    from concourse.tile import TileContext

    dt = mybir.dt
    f32 = dt.float32
    Alu = mybir.AluOpType
    Act = mybir.ActivationFunctionType

    nc = bass.Bass()

    x_t = nc.dram_tensor("x", [3, NCH, 128, TPAD], f32, kind="ExternalInput")
    pr_t = nc.dram_tensor("pr", [16, NCH, 128], f32, kind="ExternalInput")
    out_t = nc.dram_tensor("out", [NCH, 128, T], f32, kind="ExternalOutput")

    # constants embedded in the NEFF
    ident_np = np.eye(128, dtype=np.float32)
    tl = np.arange(L, dtype=np.float32) + 0.5
    tl_np = np.tile(tl, (128, 1))
    lntl_np = np.tile(np.log(tl), (128, 1))
    ident_t = nc.inline_tensor(ident_np, "ident")
    tl_t = nc.inline_tensor(tl_np, "tlc")
    lntl_t = nc.inline_tensor(lntl_np, "lntlc")

    SERW = 16 + TPAD + 16
    ser_ho = nc.dram_tensor("ser_ho", [NCH, 128, SERW], f32, kind="Internal")
    ser_qd = nc.dram_tensor("ser_qd", [NCH, 128, SERW], f32, kind="Internal")
    ser_ph = nc.dram_tensor("ser_ph", [NCH, 128, SERW], f32, kind="Internal")

    with TileContext(nc) as tc:
        with (
            tc.tile_pool(name="pers", bufs=1) as pers,
            tc.tile_pool(name="blk", bufs=1) as blk,
            tc.tile_pool(name="conv", bufs=3) as convp,
            tc.tile_pool(name="psum", bufs=2, space="PSUM") as psump,
        ):
            praw = pers.tile([128, 16 * NCH], f32, tag="praw", name="praw")
            # derived params, each [128, NCH]
            NPAR = 20
            pd = pers.tile([128, NPAR * NCH], f32, tag="pd", name="pd")
            (P_DDF, P_TBM, P_WRF, P_TBF, P_KF, P_FE, P_ETV, P_CR, P_CV,
             P_C2P, P_CP, P_VM, P_IVM, P_EPSV, P_DPHI, P_A1M, P_IB1,
             P_A2M, P_NDT, P_LNKF) = range(NPAR)
            pd2 = pers.tile([128, 2 * NCH], f32, tag="pd2", name="pd2")
            P2_IB2 = 0

            def pcol(j, c=None):
                if c is None:
                    return pd[:, j * NCH:(j + 1) * NCH]
                return pd[:, j * NCH + c:j * NCH + c + 1]

            def p2col(j, c=None):
                if c is None:
                    return pd2[:, j * NCH:(j + 1) * NCH]
                return pd2[:, j * NCH + c:j * NCH + c + 1]

            ident = pers.tile([128, 128], f32, tag="ident", name="identt")
            tlt = pers.tile([128, L], f32, tag="tlt", name="tlt")
            lntlt = pers.tile([128, L], f32, tag="lntlt", name="lntlt")
            uhw = pers.tile([128, 3 * NCH * L], f32, tag="uhw", name="uhw")  # W1 | W2h | W2p
            st = pers.tile([128, 4 * NCH], f32, tag="st", name="stt_")  # Om | Sg | V0 | V1
            # DVE scratch: slots 0..22 (incl. parity-doubled producer slots);
            # Pool scratch: slots 23..26
            sc = pers.tile([128, 27 * NCH], f32, tag="sc", name="sc")
            zeros16 = pers.tile([128, 16], f32, tag="z16", name="z16")

            raw = [blk.tile([128, 3 * NCH * TB], f32, tag="raw0", name="raw0")]
            strm = [blk.tile([128, 5 * NCH * TB], f32, tag=f"strm{i}", name=f"strm{i}") for i in range(2)]
            ser = [blk.tile([128, 4 * NCH * TB], f32, tag=f"ser{i}", name=f"ser{i}") for i in range(2)]
            pa_a = [blk.tile([128, TB], f32, tag=f"pa_a{i}", name=f"pa_a{i}") for i in range(2)]
            pa_b = [blk.tile([128, TB], f32, tag=f"pa_b{i}", name=f"pa_b{i}") for i in range(2)]
            pa_c = blk.tile([128, TB], f32, tag="pa_c", name="pa_c")

            nc.sync.dma_start(ident[:], ident_t[:, :])
            nc.sync.dma_start(tlt[:], tl_t[:, :])
            nc.sync.dma_start(lntlt[:], lntl_t[:, :])
            nc.sync.dma_start(praw[:], pr_t.rearrange("j c p -> p (j c)"))
            nc.vector.memset(zeros16[:], 0.0)

            def rawp(j):
                return praw[:, j * NCH:(j + 1) * NCH]

            ts = nc.vector.tensor_scalar
            tt = nc.vector.tensor_tensor
            stt = nc.vector.scalar_tensor_tensor
            ptt = nc.gpsimd.tensor_tensor
            pts = nc.gpsimd.tensor_scalar

            # ---- derive params ----
            def ds(dst, j, lo, hi):
                ts(dst, rawp(j), float(hi - lo), float(lo), Alu.mult, Alu.add)

            ds(pcol(P_DDF), 0, 0.0, 40.0)
            ds(pcol(P_TBM), 1, -2.0, 3.0)
            ds(pcol(P_WRF), 2, 0.0, 0.5)
            ds(pcol(P_TBF), 3, -5.0, 2.0)
            ds(pcol(P_KF), 4, 0.0, 5.0)
            ds(pcol(P_FE), 5, 0.0, 1.0)
            ds(pcol(P_ETV), 6, 0.0, 1.0)        # ET for now; *ivm below
            ds(pcol(P_CR), 7, 0.0, 1.0)
            ds(pcol(P_C2P), 8, 1e-5, 0.02)
            ds(pcol(P_CV), 9, 0.0, 0.1)
            ds(pcol(P_CP), 10, 1e-5, 0.01)
            ds(pcol(P_VM), 11, 1e-3, 500.0)
            ds(pcol(P_A1M), 12, 0.3, 20.0)
            ts(pcol(P_A1M), pcol(P_A1M), -1.0, None, Alu.add)   # alpha1 - 1
            ds(pcol(P_IB1), 13, 0.01, 5.0)
            ds(pcol(P_A2M), 14, 0.5, 13.0)
            ts(pcol(P_A2M), pcol(P_A2M), -1.0, None, Alu.add)
            ds(p2col(P2_IB2), 15, 0.15, 1.5)
            nc.vector.reciprocal(pcol(P_IVM), pcol(P_VM))
            nc.vector.reciprocal(pcol(P_IB1), pcol(P_IB1))
            nc.vector.reciprocal(p2col(P2_IB2), p2col(P2_IB2))
            tt(pcol(P_ETV), pcol(P_ETV), pcol(P_IVM), Alu.mult)
            ts(pcol(P_EPSV), pcol(P_IVM), NZ, None, Alu.mult)
            ts(pcol(P_DPHI), pcol(P_CP), -1.0, 1.0, Alu.mult, Alu.add)
            # -ddf*Tbm (bias for the melt-pot Relu) and clamped ln(Kf)
            tt(pcol(P_NDT), pcol(P_DDF), pcol(P_TBM), Alu.mult)
            ts(pcol(P_NDT), pcol(P_NDT), -1.0, None, Alu.mult)
            nc.scalar.activation(pcol(P_LNKF), pcol(P_KF), Act.Ln)
            ts(pcol(P_LNKF), pcol(P_LNKF), -80.0, None, Alu.max)

            # ---- UH weights + series prefixes: emitted after block 0 so
            # the first block's DMA/phase-A/scan work starts immediately ----
            lgt = blk.tile([128, L], f32, tag="lgt", name="lgt")
            et = blk.tile([128, L], f32, tag="et", name="et")
            ssum = blk.tile([128, 1], f32, tag="ssum", name="ssum")

            def emit_uh_and_prefixes():
                for ui, amj in enumerate([P_A1M, P_A2M]):
                    for c in range(NCH):
                        am = pcol(amj, c)
                        ib = pcol(P_IB1, c) if ui == 0 else p2col(P2_IB2, c)
                        ts(lgt[:], lntlt[:], am, None, Alu.mult)
                        stt(lgt[:], tlt[:], ib, lgt[:], Alu.mult, Alu.subtract)
                        nc.scalar.activation(et[:], lgt[:], Act.Exp, scale=-1.0)
                        nc.vector.tensor_reduce(ssum[:], et[:], mybir.AxisListType.X, Alu.add)
                        nc.vector.reciprocal(ssum[:], ssum[:])
                        ts(et[:], et[:], ssum[:], None, Alu.mult)
                        wdst = uhw[:, (ui * NCH + c) * L:(ui * NCH + c) * L + L]
                        ts(wdst, et[:], pcol(P_VM, c), None, Alu.mult)
                        if ui == 1:
                            w2p = uhw[:, (2 * NCH + c) * L:(2 * NCH + c) * L + L]
                            ts(w2p, wdst, pcol(P_CP, c), None, Alu.mult)

                for c in range(NCH):
                    nc.sync.dma_start(ser_ho[c, :, 0:16], zeros16[:])
                    nc.sync.dma_start(ser_qd[c, :, 0:16], zeros16[:])
                    nc.sync.dma_start(ser_ph[c, :, 0:16], zeros16[:])
                    nc.sync.dma_start(ser_ph[c, :, 16:17], pcol(P_EPSV, c))

            # ---- init states: Om=Sg=1e-5, V=Phi=1e-5/vm ----
            OM, SG, V0, V1 = 0, NCH, 2 * NCH, 3 * NCH
            nc.vector.memset(st[:, OM:OM + NCH], NZ)
            nc.vector.memset(st[:, SG:SG + NCH], NZ)
            ts(st[:, V0:V0 + NCH], pcol(P_IVM), NZ, None, Alu.mult)
            ts(st[:, V1:V1 + NCH], pcol(P_IVM), NZ, None, Alu.mult)

            def S(i):
                return sc[:, i * NCH:(i + 1) * NCH]

            # conv emission (interleaved under later scan blocks)
            FBS = [(f0, min(256, T - f0)) for f0 in range(0, T, 256)]

            def emit_conv(fb_idx):
                f0, F = FBS[fb_idx]
                for c in range(NCH):
                    ps = psump.tile([128, F], f32, tag="ps", name="ps")
                    first = True
                    for si, serd in enumerate([ser_ho, ser_qd, ser_ph]):
                        rhs = convp.tile([128, F + 14], f32, tag="rhs", name="rhs")
                        nc.sync.dma_start(rhs[:], serd[c, :, 2 + f0:2 + f0 + F + 14])
                        for l in range(L):
                            dg = convp.tile([128, 128], f32, tag="dg", name="dg")
                            wcol = uhw[:, (si * NCH + c) * L + l:(si * NCH + c) * L + l + 1]
                            nc.scalar.activation(dg[:], ident[:], Act.Copy, scale=wcol)
                            nc.tensor.matmul(
                                ps[:, 0:F], dg[:], rhs[:, 14 - l:14 - l + F],
                                start=first, stop=(si == 2 and l == L - 1),
                            )
                            first = False
                    ot = convp.tile([128, F], f32, tag="ot", name="ot")
                    nc.scalar.copy(ot[:], ps[:, 0:F])
                    nc.sync.dma_start(out_t[c, :, f0:f0 + F], ot[:])

            conv_after = {}
            for fb, (f0, F) in enumerate(FBS):
                conv_after.setdefault(min((f0 + F + TB - 1) // TB - 1, NB - 1),
                                      []).append(fb)

            for b in range(NB):
                pa = b % 2
                t0 = b * TB
                rw, sm, se = raw[0], strm[pa], ser[pa]
                for ch in range(3):
                    for c in range(NCH):
                        nc.sync.dma_start(
                            rw[:, (ch * NCH + c) * TB:(ch * NCH + c + 1) * TB],
                            x_t[ch, c, :, t0:t0 + TB],
                        )

                def rch(ch, c):
                    return rw[:, (ch * NCH + c) * TB:(ch * NCH + c) * TB + TB]

                def sch(s, c):
                    return sm[:, (s * NCH + c) * TB:(s * NCH + c) * TB + TB]

                # ---- phase A: streams PF(0) RPv(1) S(2) R(3) MP(4) ----
                for c in range(NCH):
                    Pc, Tc, Ec = rch(0, c), rch(1, c), rch(2, c)
                    stt(sch(3, c), Tc, 0.0, Pc, Alu.is_ge, Alu.mult)   # rain (DVE)
                    stt(sch(2, c), Tc, 0.0, Pc, Alu.is_lt, Alu.mult)   # snow (DVE)
                    ts(pa_c[:], Tc, pcol(P_TBM, c), pcol(P_DDF, c), Alu.subtract, Alu.mult)
                    ts(sch(4, c), pa_c[:], 0.0, None, Alu.max)          # melt pot (DVE)
                    paa, pab = pa_a[c % 2], pa_b[c % 2]
                    pts(paa[:], Tc, pcol(P_TBF, c), -1.0, Alu.subtract, Alu.mult)  # Pool
                    pts(paa[:], paa[:], NZ, None, Alu.max)                         # Pool
                    nc.scalar.activation(pab[:], paa[:], Act.Ln)        # ACT
                    nc.scalar.activation(paa[:], pab[:], Act.Exp, scale=pcol(P_FE, c))
                    ts(sch(0, c), paa[:], pcol(P_KF, c), None, Alu.mult)  # freeze pot
                    nc.scalar.activation(sch(1, c), Ec, Act.Copy,       # pet/vm (ACT)
                                         scale=pcol(P_ETV, c))

                sm4 = sm[:].rearrange("p (s c t) -> p t (s c)", s=5, c=NCH, t=TB)
                se4 = se[:].rearrange("p (s c t) -> p t (s c)", s=4, c=NCH, t=TB)
                sep4 = ser[1 - pa][:].rearrange("p (s c t) -> p t (s c)", s=4, c=NCH, t=TB)

                # ---- phase B: pipelined scan ----
                # DVE: snow chain + V-clip + aet (13 ops/step)
                # Pool: soil arithmetic chain (11 ops/step) writing the
                #       h0/h1/h2/Vp series slots consumed by batch-post.
                SL_H0, SL_H1, SL_H2, SL_VP = 0, NCH, 2 * NCH, 3 * NCH
                for t in range(TB):
                    k = t0 + t
                    PFt = sm4[:, t, 0:NCH]
                    RPt = sm4[:, t, NCH:2 * NCH]
                    Stt = sm4[:, t, 2 * NCH:3 * NCH]
                    Rtt = sm4[:, t, 3 * NCH:4 * NCH]
                    MPt = sm4[:, t, 4 * NCH:5 * NCH]
                    Vcur = st[:, V0 + (k % 2) * NCH:V0 + (k % 2) * NCH + NCH]
                    if t == 0:
                        VpPrev = sep4[:, TB - 1, SL_VP:SL_VP + NCH] if b > 0 else None
                    else:
                        VpPrev = se4[:, t - 1, SL_VP:SL_VP + NCH]

                    # -- DVE snow chain (iteration k) --
                    tt(S(0), PFt, st[:, OM:OM + NCH], Alu.min)             # f
                    tt(S(2), st[:, OM:OM + NCH], S(0), Alu.subtract)       # Oma
                    tt(S(3), st[:, SG:SG + NCH], S(0), Alu.add)            # Sga
                    tt(S(4), S(3), Stt, Alu.add)                           # Sgs
                    tt(S(5), MPt, S(4), Alu.min)                           # m
                    tt(st[:, SG:SG + NCH], S(4), S(5), Alu.subtract)       # Sg'
                    tt(S(6), S(2), S(5), Alu.add)                          # wa
                    tt(S(7), S(6), Rtt, Alu.add)                           # w
                    tt(S(8), pcol(P_WRF), st[:, SG:SG + NCH], Alu.mult)    # ret
                    tt(st[:, OM:OM + NCH], S(7), S(8), Alu.min)            # Om'
                    tt(S(9), S(7), st[:, OM:OM + NCH], Alu.subtract)       # avail = w - Om'
                    # V'(k-1): clip previous step's Vp into the V state slot
                    if VpPrev is not None:
                        stt(Vcur, VpPrev, 1.0, pcol(P_EPSV), Alu.min, Alu.max)
                    tt(S(1), RPt, Vcur, Alu.min)                           # aet
                    # -- Pool soil chain (iteration k) --
                    ptt(S(10), S(9), pcol(P_IVM), Alu.mult)                # Av
                    ptt(S(11), pcol(P_CR), Vcur, Alu.mult)                 # u
                    ptt(se4[:, t, SL_H0:SL_H0 + NCH], S(11), S(10), Alu.mult)   # h0
                    ptt(S(12), S(10), se4[:, t, SL_H0:SL_H0 + NCH], Alu.subtract)  # i1
                    ptt(S(13), Vcur, S(12), Alu.add)                       # x1
                    ptt(S(14), S(13), S(1), Alu.subtract)                  # x2
                    ptt(S(15), Vcur, Vcur, Alu.mult)                       # v2
                    ptt(se4[:, t, SL_H1:SL_H1 + NCH], pcol(P_C2P), S(15), Alu.mult)  # h1
                    ptt(se4[:, t, SL_H2:SL_H2 + NCH], pcol(P_CV), Vcur, Alu.mult)    # h2
                    ptt(S(16), S(14), se4[:, t, SL_H1:SL_H1 + NCH], Alu.subtract)    # x3
                    ptt(se4[:, t, SL_VP:SL_VP + NCH], S(16),
                        se4[:, t, SL_H2:SL_H2 + NCH], Alu.subtract)        # Vp

                # ---- batch-post: ho, phi (TTS), series DMA out ----
                seb = se[:]
                for c in range(NCH):
                    h0b = seb[:, c * TB:c * TB + TB]
                    h1b = seb[:, (NCH + c) * TB:(NCH + c) * TB + TB]
                    h2b = seb[:, (2 * NCH + c) * TB:(2 * NCH + c) * TB + TB]
                    vpb = seb[:, (3 * NCH + c) * TB:(3 * NCH + c) * TB + TB]
                    pts(pa_c[:], vpb, 1.0, 0.0, Alu.subtract, Alu.max)     # ovf (Pool)
                    ptt(h0b, h0b, pa_c[:], Alu.add)                        # ho (Pool, in-place)
                    if b == 0:
                        phi_init = pcol(P_EPSV, c)
                    else:
                        phi_init = ser[1 - pa][:, (NCH + c) * TB + TB - 1:
                                               (NCH + c) * TB + TB]
                    nc.vector.tensor_tensor_scan(
                        h1b, pcol(P_DPHI, c).broadcast_to((128, TB)), h1b,
                        phi_init, Alu.mult, Alu.add)                       # phi (in-place over h1)
                    ts(h1b, h1b, pcol(P_EPSV, c), None, Alu.max)           # eps floor
                    nc.sync.dma_start(ser_ho[c, :, 16 + t0:16 + t0 + TB], h0b)
                    nc.sync.dma_start(ser_qd[c, :, 16 + t0:16 + t0 + TB], h2b)
                    nc.sync.dma_start(ser_ph[c, :, 17 + t0:17 + t0 + TB], h1b)

                if b == 0:
                    emit_uh_and_prefixes()
                for fb in conv_after.get(b, []):
                    emit_conv(fb)

    _split_multi_waits(nc)
    return nc


def _split_multi_waits(nc):
    """This container's walrus codegen accepts at most ONE sync wait per
    instruction; Tile emits several.  Hoist the excess onto same-engine
    NoOp carriers inserted immediately before."""
    from bass_rust import InstNoOp, SyncInfo

    cnt = 0
    for f in nc.m.functions:
        for bb in f.blocks:
            out = []
            changed = False
            for ins in bb.instructions:
                si = ins.sync_info
                w = list(si.on_wait) if si is not None and si.on_wait else []
                if len(w) > 1:
                    for extra in w[:-1]:
                        cnt += 1
                        nop = InstNoOp(name=f"WQ-{cnt}", engine=ins.engine)
                        nop.sync_info = SyncInfo(on_wait=[extra], on_update=[])
                        out.append(nop)
                    si.on_wait = [w[-1]]
                    changed = True
                out.append(ins)
            if changed:
                bb.instructions = out


def _get_program():
    if "nc" not in _CACHE:
        _CACHE["nc"] = _build_program()
    return _CACHE["nc"]


def kernel(x_phy: np.ndarray, params: np.ndarray) -> np.ndarray:
    from concourse.bass_utils import run_bass_kernel_spmd

    nc = _get_program()

    x_phy = np.ascontiguousarray(x_phy, dtype=np.float32)
    params = np.ascontiguousarray(params, dtype=np.float32)

    GPAD = NCORES * GC
    xp = np.zeros((TPAD, GPAD, 3), np.float32)
    xp[:T, :G] = x_phy
    pp = np.full((GPAD, 16), 0.5, np.float32)
    pp[:G] = params

    in_maps = []
    for k in range(NCORES):
        g0 = k * GC
        xk = np.ascontiguousarray(
            xp[:, g0:g0 + GC].transpose(2, 1, 0).reshape(3, NCH, 128, TPAD))
        pk = np.ascontiguousarray(
            pp[g0:g0 + GC].reshape(NCH, 128, 16).transpose(2, 0, 1))
        in_maps.append({"x": xk, "pr": pk})

    res = run_bass_kernel_spmd(nc, in_maps, core_ids=list(range(NCORES)))

    out = np.empty((T, G), np.float32)
    for k in range(NCORES):
        o = res.results[k]["out"]            # [NCH,128,T]
        g0 = k * GC
        hi = min(g0 + GC, G)
        flat = o.transpose(2, 0, 1).reshape(T, GC)
        out[:, g0:hi] = flat[:, :hi - g0]
    return out



# revision 6
# speedup vs baseline: 1.3859x; 1.3859x over previous
"""Exphydro (snow + 2-bucket soil + gamma-UH routing) Trainium2 Bass kernel.

Contract: kernel(x_phy [1095,15000,3] f32, params [15000,16] f32) -> [1095,15000] f32.
Shards the grid dim across 8 NeuronCores (1875 -> padded 1920 per core).

v2 architecture (vs v1's 24 serial ops/step):
  - snow recurrence rewritten in (pre-relu Sg, U=Om+Sg) coordinates: 6 Pool
    stt ops/step; avail = U + P - U' recovered in batch per block.
  - vadose recurrence grouped as V' = clip(V(C - c2p V) + Av - min(RP,V)):
    7 DVE ops/step, running one block behind the snow scan.
  - all series terms (ho, h1, qd2=h2+cp*phi, phi via tensor_tensor_scan)
    computed in batch [128, 128] ops, mostly on ACT with per-chunk [128,1]
    scale/bias vectors.
  - gamma-UH conv: 2 series x 15 taps as diag matmuls in fp32r (1 cyc/row at
    F>=256) on the PE, plus a 71-wide tail block on DVE.
"""
import numpy as np

T = 1095
TB = 128
NB = 9
TPAD = TB * NB           # 1152
G = 15000
NCORES = 8
GC = 1920                # padded grid per core
NCH = 15                 # chunks of 128 per core
L = 15                   # UH length
NZ = 1e-5

_CACHE = {}

# conv F-blocks: two PE groups of 512, one DVE tail of 71
FBS = [(0, 512), (512, 512), (1024, 71)]
# iteration -> list of (fb_index, chunk) PE conv units emitted there
CONV_SCHED = {
    4: [(0, c) for c in range(0, 4)],
    5: [(0, c) for c in range(4, 8)],
    6: [(0, c) for c in range(8, 12)],
    7: [(0, c) for c in range(12, 15)],
    8: [(1, c) for c in range(0, 5)],
    9: [(1, c) for c in range(5, 10)],
}
CONV_TAIL_PE = [(1, c) for c in range(10, 15)]


def _build_program():
    import concourse.bass as bass
    import concourse.mybir as mybir
    from concourse.tile import TileContext

    dt = mybir.dt
    f32 = dt.float32
    f32r = dt.float32r
    Alu = mybir.AluOpType
    Act = mybir.ActivationFunctionType

    nc = bass.Bass()

    x_t = nc.dram_tensor("x", [3, NCH, 128, TPAD], f32, kind="ExternalInput")
    pr_t = nc.dram_tensor("pr", [16, NCH, 128], f32, kind="ExternalInput")
    out_t = nc.dram_tensor("out", [NCH, 128, T], f32, kind="ExternalOutput")

    ident_np = np.eye(128, dtype=np.float32)
    tl = np.arange(L, dtype=np.float32) + 0.5
    tl_np = np.tile(tl, (128, 1))
    lntl_np = np.tile(np.log(tl), (128, 1))
    ident_t = nc.inline_tensor(ident_np, "ident")
    tl_t = nc.inline_tensor(tl_np, "tlc")
    lntl_t = nc.inline_tensor(lntl_np, "lntlc")

    SERW = 16 + TPAD + 16
    ser_t = nc.dram_tensor("ser", [2, NCH, 128, SERW], f32, kind="Internal")

    with TileContext(nc) as tc:
        with (
            tc.tile_pool(name="pers", bufs=1) as pers,
            tc.tile_pool(name="blk", bufs=1) as blk,
            tc.tile_pool(name="pv", bufs=3) as pvp,
            tc.tile_pool(name="conv", bufs=2) as convp,
            tc.tile_pool(name="psum", bufs=2, space="PSUM") as psump,
        ):
            praw = pers.tile([128, 16 * NCH], f32, tag="praw", name="praw")
            NPAR = 21
            pd = pers.tile([128, NPAR * NCH], f32, tag="pd", name="pd")
            (P_W1P, P_C2P, P_EPSV, P_TBF, P_FE, P_LNKF, P_DDF, P_NDT,
             P_NETV, P_IVM, P_NCR, P_OMCV, P_CR, P_CV, P_CP, P_DPHI,
             P_VM, P_A1M, P_IB1, P_A2M, P_IB2) = range(NPAR)

            def pcol(j, c=None):
                if c is None:
                    return pd[:, j * NCH:(j + 1) * NCH]
                return pd[:, j * NCH + c:j * NCH + c + 1]

            ident = pers.tile([128, 128], f32, tag="ident", name="identt")
            tlt = pers.tile([128, L], f32, tag="tlt", name="tlt")
            lntlt = pers.tile([128, L], f32, tag="lntlt", name="lntlt")
            uhw = pers.tile([128, 2 * NCH * L], f32, tag="uhw", name="uhw")
            zeros16 = pers.tile([128, 16], f32, tag="z16", name="z16")
            cbias = pers.tile([128, 2], f32, tag="cbias", name="cbias")

            # scan states / scratch
            y2st = pers.tile([128, NCH], f32, tag="y2st", name="y2st")
            scp = pers.tile([128, 3 * NCH], f32, tag="scp", name="scp")  # y1|q|T4
            scv = pers.tile([128, 5 * NCH], f32, tag="scv", name="scv")  # z1|z2|z3|na|z4
            uss = pers.tile([128, NCH * (TB + 1)], f32, tag="uss", name="uss")
            vss = pers.tile([128, NCH * (TB + 1)], f32, tag="vss", name="vss")
            phs = pers.tile([128, NCH * (TB + 1)], f32, tag="phs", name="phs")

            raw = [blk.tile([128, 3 * NCH * TB], f32, tag=f"raw{i}", name=f"raw{i}")
                   for i in range(2)]
            sst = [blk.tile([128, 2 * NCH * TB], f32, tag=f"sst{i}", name=f"sst{i}")
                   for i in range(2)]
            vst = [blk.tile([128, 3 * NCH * TB], f32, tag=f"vst{i}", name=f"vst{i}")
                   for i in range(2)]
            ups = blk.tile([128, NCH * TB], f32, tag="ups", name="ups")
            vps = blk.tile([128, NCH * TB], f32, tag="vps", name="vps")
            paa = [blk.tile([128, TB], f32, tag=f"paa{i}", name=f"paa{i}") for i in range(2)]
            pab = [blk.tile([128, TB], f32, tag=f"pab{i}", name=f"pab{i}") for i in range(2)]
            pac = [blk.tile([128, TB], f32, tag=f"pac{i}", name=f"pac{i}") for i in range(2)]

            nc.sync.dma_start(ident[:], ident_t[:, :])
            nc.sync.dma_start(tlt[:], tl_t[:, :])
            nc.sync.dma_start(lntlt[:], lntl_t[:, :])
            nc.sync.dma_start(praw[:], pr_t.rearrange("j c p -> p (j c)"))
            nc.vector.memset(zeros16[:], 0.0)
            nc.vector.memset(cbias[:, 0:1], NZ)
            nc.vector.memset(cbias[:, 1:2], -1.0)

            def rawp(j):
                return praw[:, j * NCH:(j + 1) * NCH]

            ts = nc.vector.tensor_scalar
            tt = nc.vector.tensor_tensor
            stt = nc.vector.scalar_tensor_tensor
            pstt = nc.gpsimd.scalar_tensor_tensor
            pts = nc.gpsimd.tensor_scalar
            act = nc.scalar.activation

            # ---- derive params ----
            def ds(dst, j, lo, hi):
                ts(dst, rawp(j), float(hi - lo), float(lo), Alu.mult, Alu.add)

            ds(pcol(P_DDF), 0, 0.0, 40.0)
            ds(pcol(P_NDT), 1, -2.0, 3.0)                       # Tbm for now
            tt(pcol(P_NDT), pcol(P_DDF), pcol(P_NDT), Alu.mult)
            ts(pcol(P_NDT), pcol(P_NDT), -1.0, None, Alu.mult)  # -ddf*Tbm
            ds(pcol(P_W1P), 2, 1.0, 1.5)                        # 1 + wrf
            ds(pcol(P_TBF), 3, -5.0, 2.0)
            ds(pcol(P_LNKF), 4, 0.0, 5.0)                       # Kf for now
            act(pcol(P_LNKF), pcol(P_LNKF), Act.Ln)
            ts(pcol(P_LNKF), pcol(P_LNKF), -80.0, None, Alu.max)
            ds(pcol(P_FE), 5, 0.0, 1.0)
            ds(pcol(P_NETV), 6, 0.0, 1.0)                       # ETe for now
            ds(pcol(P_NCR), 7, 0.0, 1.0)                        # cr for now
            ds(pcol(P_C2P), 8, 1e-5, 0.02)
            ds(pcol(P_CV), 9, 0.0, 0.1)
            ds(pcol(P_CP), 10, 1e-5, 0.01)
            ds(pcol(P_VM), 11, 1e-3, 500.0)
            nc.vector.reciprocal(pcol(P_IVM), pcol(P_VM))
            tt(pcol(P_NETV), pcol(P_NETV), pcol(P_IVM), Alu.mult)
            ts(pcol(P_NETV), pcol(P_NETV), -1.0, None, Alu.mult)  # -ETe*ivm
            ts(pcol(P_EPSV), pcol(P_IVM), NZ, None, Alu.mult)
            ts(pcol(P_CR), pcol(P_NCR), 1.0, None, Alu.mult)      # cr
            ts(pcol(P_NCR), pcol(P_NCR), -1.0, None, Alu.mult)    # -cr
            ts(pcol(P_OMCV), pcol(P_CV), -1.0, 1.0, Alu.mult, Alu.add)  # 1-cv
            ts(pcol(P_DPHI), pcol(P_CP), -1.0, 1.0, Alu.mult, Alu.add)  # 1-cp
            ds(pcol(P_A1M), 12, 0.3, 20.0)
            ts(pcol(P_A1M), pcol(P_A1M), -1.0, None, Alu.add)
            ds(pcol(P_IB1), 13, 0.01, 5.0)
            nc.vector.reciprocal(pcol(P_IB1), pcol(P_IB1))
            ds(pcol(P_A2M), 14, 0.5, 13.0)
            ts(pcol(P_A2M), pcol(P_A2M), -1.0, None, Alu.add)
            ds(pcol(P_IB2), 15, 0.15, 1.5)
            nc.vector.reciprocal(pcol(P_IB2), pcol(P_IB2))

            # ---- state init ----
            nc.vector.memset(y2st[:], NZ)                        # Sg0 = NZ
            us4 = uss[:].rearrange("p (c t) -> p t c", c=NCH, t=TB + 1)
            vs4 = vss[:].rearrange("p (c t) -> p t c", c=NCH, t=TB + 1)
            ph4 = phs[:].rearrange("p (c t) -> p t c", c=NCH, t=TB + 1)
            nc.gpsimd.memset(us4[:, 0, :], 2 * NZ)               # U0 = Om0+Sg0
            ts(vs4[:, 0, :], pcol(P_EPSV), 0.0, None, Alu.add)   # V0 = eps/vm
            ts(ph4[:, 0, :], pcol(P_EPSV), 0.0, None, Alu.add)   # phi0

            # ---- UH weights + series zero prefixes (emitted after block 0) ----
            lgt = blk.tile([128, L], f32, tag="lgt", name="lgt")
            et = blk.tile([128, L], f32, tag="et", name="et")
            ssum = blk.tile([128, 1], f32, tag="ssum", name="ssum")

            def emit_uh_and_prefixes():
                for ui, amj in enumerate([P_A1M, P_A2M]):
                    for c in range(NCH):
                        am = pcol(amj, c)
                        ib = pcol(P_IB1, c) if ui == 0 else pcol(P_IB2, c)
                        ts(lgt[:], lntlt[:], am, None, Alu.mult)
                        stt(lgt[:], tlt[:], ib, lgt[:], Alu.mult, Alu.subtract)
                        act(et[:], lgt[:], Act.Exp, scale=-1.0)
                        nc.vector.tensor_reduce(ssum[:], et[:], mybir.AxisListType.X, Alu.add)
                        nc.vector.reciprocal(ssum[:], ssum[:])
                        ts(et[:], et[:], ssum[:], None, Alu.mult)
                        wdst = uhw[:, (ui * NCH + c) * L:(ui * NCH + c) * L + L]
                        ts(wdst, et[:], pcol(P_VM, c), None, Alu.mult)
                for c in range(NCH):
                    for si in range(2):
                        nc.sync.dma_start(ser_t[si, c, :, 0:16], zeros16[:])

            # ---- conv unit (PE, fp32r diag matmuls) ----
            def emit_conv_pe(fbi, c):
                f0, F = FBS[fbi]
                rhs = convp.tile([128, 2 * (F + 14)], f32, tag="rhs", name="rhs")
                nc.sync.dma_start(
                    rhs[:].rearrange("p (s t) -> p s t", s=2, t=F + 14),
                    ser_t[:, c, :, 2 + f0:2 + f0 + F + 14].rearrange("s p t -> p s t"))
                ps = psump.tile([128, F], f32, tag="ps", name="ps")
                for s in range(2):
                    for l in range(L):
                        dg = convp.tile([128, 128], f32, tag="dg", name="dg")
                        wcol = uhw[:, (s * NCH + c) * L + l:(s * NCH + c) * L + l + 1]
                        act(dg[:], ident[:], Act.Copy, scale=wcol)
                        nc.tensor.matmul(
                            ps[:, 0:F], dg[:].bitcast(f32r),
                            rhs[:, s * (F + 14) + 14 - l:s * (F + 14) + 14 - l + F].bitcast(f32r),
                            start=(s == 0 and l == 0), stop=(s == 1 and l == L - 1))
                ot = convp.tile([128, F], f32, tag="ot", name="ot")
                nc.scalar.copy(ot[:], ps[:, 0:F])
                nc.sync.dma_start(out_t[c, :, f0:f0 + F], ot[:])

            # ---- conv tail unit (DVE stt accumulate) ----
            def emit_conv_dve(fbi, c):
                f0, F = FBS[fbi]
                rhs = convp.tile([128, 2 * (F + 14)], f32, tag="rhs2", name="rhs2")
                nc.sync.dma_start(
                    rhs[:].rearrange("p (s t) -> p s t", s=2, t=F + 14),
                    ser_t[:, c, :, 2 + f0:2 + f0 + F + 14].rearrange("s p t -> p s t"))
                acc = convp.tile([128, F], f32, tag="acc", name="acc")
                first = True
                for s in range(2):
                    for l in range(L):
                        wcol = uhw[:, (s * NCH + c) * L + l:(s * NCH + c) * L + l + 1]
                        rsl = rhs[:, s * (F + 14) + 14 - l:s * (F + 14) + 14 - l + F]
                        if first:
                            ts(acc[:], rsl, wcol, None, Alu.mult)
                            first = False
                        else:
                            stt(acc[:], rsl, wcol, acc[:], Alu.mult, Alu.add)
                nc.sync.dma_start(out_t[c, :, f0:f0 + F], acc[:])

            # ---- phase A: derive streams for block b ----
            def emit_phase_a(b):
                pa = b % 2
                rw, ss, vs = raw[pa], sst[pa], vst[pa]
                for c in range(NCH):
                    Pc = rw[:, (c * 3 + 0) * TB:(c * 3 + 0) * TB + TB]
                    Tc = rw[:, (c * 3 + 1) * TB:(c * 3 + 1) * TB + TB]
                    Ec = rw[:, (c * 3 + 2) * TB:(c * 3 + 2) * TB + TB]
                    a_, b_, c_ = paa[c % 2], pab[c % 2], pac[c % 2]
                    PFd = ss[:, (0 * NCH + c) * TB:(0 * NCH + c) * TB + TB]
                    Ad = ss[:, (1 * NCH + c) * TB:(1 * NCH + c) * TB + TB]
                    NRd = vs[:, (2 * NCH + c) * TB:(2 * NCH + c) * TB + TB]
                    act(a_[:], Tc, Act.Relu, scale=-1.0, bias=pcol(P_TBF, c))
                    act(b_[:], a_[:], Act.Ln, bias=cbias[:, 0:1])
                    act(PFd, b_[:], Act.Exp, scale=pcol(P_FE, c), bias=pcol(P_LNKF, c))
                    act(a_[:], Tc, Act.Relu, scale=pcol(P_DDF, c), bias=pcol(P_NDT, c))
                    act(NRd, Ec, Act.Copy, scale=pcol(P_NETV, c))
                    stt(c_[:], Tc, 0.0, Pc, Alu.is_lt, Alu.mult)      # snow
                    tt(Ad, c_[:], a_[:], Alu.subtract)                # A = S - MP

            # ---- post-snow: avail -> Av, C streams for block b ----
            def emit_post_snow(b):
                pa = b % 2
                vs = vst[pa]
                for c in range(NCH):
                    c_ = pac[c % 2]
                    AVd = vs[:, (0 * NCH + c) * TB:(0 * NCH + c) * TB + TB]
                    Cd = vs[:, (1 * NCH + c) * TB:(1 * NCH + c) * TB + TB]
                    tt(c_[:], ups[:, c * TB:c * TB + TB],
                       uss[:, c * (TB + 1) + 1:c * (TB + 1) + 1 + TB], Alu.subtract)
                    act(AVd, c_[:], Act.Copy, scale=pcol(P_IVM, c))
                    act(Cd, AVd, Act.Identity, scale=pcol(P_NCR, c), bias=pcol(P_OMCV, c))

            # ---- post-V: series ho/qd2 for block bb, DMA to ser ----
            def emit_post_v(bb):
                t0 = bb * TB
                vsb = vst[bb % 2]
                for c in range(NCH):
                    base = c * (TB + 1)
                    VSh = vss[:, base:base + TB]
                    AVc = vsb[:, (0 * NCH + c) * TB:(0 * NCH + c) * TB + TB]
                    t_ovf = pvp.tile([128, TB], f32, tag="tovf", name="tovf")
                    t_v2 = pvp.tile([128, TB], f32, tag="tv2", name="tv2")
                    t_h1 = pvp.tile([128, TB], f32, tag="th1", name="th1")
                    t_cpp = pvp.tile([128, TB], f32, tag="tcpp", name="tcpp")
                    t_m1 = pvp.tile([128, TB], f32, tag="tm1", name="tm1")
                    hq = pvp.tile([128, 2 * TB], f32, tag="hq", name="hq")
                    act(t_ovf[:], vps[:, c * TB:c * TB + TB], Act.Relu, bias=cbias[:, 1:2])
                    act(t_v2[:], VSh, Act.Square)
                    act(t_h1[:], t_v2[:], Act.Copy, scale=pcol(P_C2P, c))
                    if bb > 0:
                        ts(phs[:, base:base + 1], phs[:, base + TB:base + TB + 1],
                           0.0, None, Alu.add)
                    nc.vector.tensor_tensor_scan(
                        phs[:, base + 1:base + 1 + TB],
                        pcol(P_DPHI, c).broadcast_to((128, TB)), t_h1[:],
                        phs[:, base:base + 1], Alu.mult, Alu.add)
                    act(t_cpp[:], phs[:, base:base + TB], Act.Copy, scale=pcol(P_CP, c))
                    tt(t_m1[:], VSh, AVc, Alu.mult)
                    stt(hq[:, 0:TB], t_m1[:], pcol(P_CR, c), t_ovf[:], Alu.mult, Alu.add)
                    stt(hq[:, TB:2 * TB], VSh, pcol(P_CV, c), t_cpp[:], Alu.mult, Alu.add)
                    nc.sync.dma_start(
                        ser_t[:, c, :, 16 + t0:16 + t0 + TB].rearrange("s p t -> p s t"),
                        hq[:].rearrange("p (s t) -> p s t", s=2, t=TB))

            # ---- raw input DMA for block b ----
            def emit_raw_dma(b):
                pa = b % 2
                t0 = b * TB
                for c in range(NCH):
                    nc.sync.dma_start(
                        raw[pa][:, c * 3 * TB:(c + 1) * 3 * TB].rearrange(
                            "p (ch t) -> p ch t", ch=3, t=TB),
                        x_t[:, c, :, t0:t0 + TB].rearrange("ch p t -> p ch t"))

            ups4 = ups[:].rearrange("p (c t) -> p t c", c=NCH, t=TB)
            vps4 = vps[:].rearrange("p (c t) -> p t c", c=NCH, t=TB)

            def S(i):
                return scp[:, i * NCH:(i + 1) * NCH]

            def Z(i):
                return scv[:, i * NCH:(i + 1) * NCH]

            emit_raw_dma(0)

            for b in range(NB + 1):
                if b < NB:
                    emit_phase_a(b)
                    if b + 1 < NB:
                        emit_raw_dma(b + 1)
                    if b >= 1:
                        pts(us4[:, 0, :], us4[:, TB, :], 0.0, None, Alu.add)
                if b >= 2:
                    ts(vs4[:, 0, :], vs4[:, TB, :], 0.0, None, Alu.add)

                sm = (sst[b % 2][:].rearrange("p (s c t) -> p t (s c)", s=2, c=NCH, t=TB)
                      if b < NB else None)
                pv = (raw[b % 2][:].rearrange("p (c s t) -> p t s c", c=NCH, s=3, t=TB)
                      if b < NB else None)
                vm4 = (vst[(b - 1) % 2][:].rearrange("p (s c t) -> p t (s c)", s=3, c=NCH, t=TB)
                       if b >= 1 else None)

                for t in range(TB):
                    if b < NB:
                        PFt = sm[:, t, 0:NCH]
                        At = sm[:, t, NCH:2 * NCH]
                        Pt = pv[:, t, 0, :]
                        Ut = us4[:, t, :]
                        pstt(S(0), y2st[:], 0.0, PFt, Alu.max, Alu.add)       # y1
                        pstt(S(1), S(0), 0.0, Ut, Alu.max, Alu.min)           # q
                        pstt(y2st[:], S(1), 1.0, At, Alu.mult, Alu.add)       # y2'
                        pstt(S(2), y2st[:], 0.0, pcol(P_W1P), Alu.max, Alu.mult)  # T4
                        pstt(ups4[:, t, :], Ut, 1.0, Pt, Alu.mult, Alu.add)   # UP
                        pstt(us4[:, t + 1, :], ups4[:, t, :], 0.0, S(2),
                             Alu.max, Alu.min)                                # U'
                    if b >= 1:
                        AVt = vm4[:, t, 0:NCH]
                        Ct = vm4[:, t, NCH:2 * NCH]
                        NRt = vm4[:, t, 2 * NCH:3 * NCH]
                        Vt = vs4[:, t, :]
                        tt(Z(0), pcol(P_C2P), Vt, Alu.mult)                   # z1
                        tt(Z(1), Ct, Z(0), Alu.subtract)                      # z2
                        tt(Z(2), Vt, Z(1), Alu.mult)                          # z3
                        stt(Z(3), Vt, -1.0, NRt, Alu.mult, Alu.max)           # na
                        tt(Z(4), Z(2), AVt, Alu.add)                          # z4
                        tt(vps4[:, t, :], Z(4), Z(3), Alu.add)                # Vp
                        stt(vs4[:, t + 1, :], vps4[:, t, :], 1.0, pcol(P_EPSV),
                            Alu.min, Alu.max)                                 # V'

                if b < NB:
                    emit_post_snow(b)
                if b >= 1:
                    emit_post_v(b - 1)
                if b == 0:
                    emit_uh_and_prefixes()
                for fbi, c in CONV_SCHED.get(b, []):
                    emit_conv_pe(fbi, c)

            for fbi, c in CONV_TAIL_PE:
                emit_conv_pe(fbi, c)
            for c in range(NCH):
                emit_conv_dve(2, c)

    _split_multi_waits(nc)
    return nc


def _split_multi_waits(nc):
    """This container's walrus codegen accepts at most ONE sync wait per
    instruction; Tile emits several.  Hoist the excess onto same-engine
    NoOp carriers inserted immediately before."""
    from bass_rust import InstNoOp, SyncInfo

    cnt = 0
    for f in nc.m.functions:
        for bb in f.blocks:
            out = []
            changed = False
            for ins in bb.instructions:
                si = ins.sync_info
                w = list(si.on_wait) if si is not None and si.on_wait else []
                if len(w) > 1:
                    for extra in w[:-1]:
                        cnt += 1
                        nop = InstNoOp(name=f"WQ-{cnt}", engine=ins.engine)
                        nop.sync_info = SyncInfo(on_wait=[extra], on_update=[])
                        out.append(nop)
                    si.on_wait = [w[-1]]
                    changed = True
                out.append(ins)
            if changed:
                bb.instructions = out


def _get_program():
    if "nc" not in _CACHE:
        _CACHE["nc"] = _build_program()
    return _CACHE["nc"]


def kernel(x_phy: np.ndarray, params: np.ndarray) -> np.ndarray:
    from concourse.bass_utils import run_bass_kernel_spmd

    nc = _get_program()

    x_phy = np.ascontiguousarray(x_phy, dtype=np.float32)
    params = np.ascontiguousarray(params, dtype=np.float32)

    GPAD = NCORES * GC
    xp = np.zeros((TPAD, GPAD, 3), np.float32)
    xp[:T, :G] = x_phy
    pp = np.full((GPAD, 16), 0.5, np.float32)
    pp[:G] = params

    in_maps = []
    for k in range(NCORES):
        g0 = k * GC
        xk = np.ascontiguousarray(
            xp[:, g0:g0 + GC].transpose(2, 1, 0).reshape(3, NCH, 128, TPAD))
        pk = np.ascontiguousarray(
            pp[g0:g0 + GC].reshape(NCH, 128, 16).transpose(2, 0, 1))
        in_maps.append({"x": xk, "pr": pk})

    res = run_bass_kernel_spmd(nc, in_maps, core_ids=list(range(NCORES)))

    out = np.empty((T, G), np.float32)
    for k in range(NCORES):
        o = res.results[k]["out"]            # [NCH,128,T]
        g0 = k * GC
        hi = min(g0 + GC, G)
        flat = o.transpose(2, 0, 1).reshape(T, GC)
        out[:, g0:hi] = flat[:, :hi - g0]
    return out


# revision 7
# speedup vs baseline: 1.6512x; 1.1915x over previous
"""Exphydro (snow + 2-bucket soil + gamma-UH routing) Trainium2 Bass kernel.

Contract: kernel(x_phy [1095,15000,3] f32, params [15000,16] f32) -> [1095,15000] f32.
Shards the grid dim across 8 NeuronCores (1875 -> padded 1920 per core).

v2 architecture (vs v1's 24 serial ops/step):
  - snow recurrence rewritten in (pre-relu Sg, U=Om+Sg) coordinates: 6 Pool
    stt ops/step; avail = U + P - U' recovered in batch per block.
  - vadose recurrence grouped as V' = clip(V(C - c2p V) + Av - min(RP,V)):
    7 DVE ops/step, running one block behind the snow scan.
  - all series terms (ho, h1, qd2=h2+cp*phi, phi via tensor_tensor_scan)
    computed in batch [128, 128] ops, mostly on ACT with per-chunk [128,1]
    scale/bias vectors.
  - gamma-UH conv: 2 series x 15 taps as diag matmuls in fp32r (1 cyc/row at
    F>=256) on the PE, plus a 71-wide tail block on DVE.
"""
import numpy as np

T = 1095
TB = 128
NB = 9
TPAD = TB * NB           # 1152
G = 15000
NCORES = 8
GC = 1920                # padded grid per core
NCH = 15                 # chunks of 128 per core
L = 15                   # UH length
NZ = 1e-5

_CACHE = {}

# conv F-blocks: two PE groups of 512, one DVE tail of 71
FBS = [(0, 512), (512, 512), (1024, 71)]
# iteration -> list of (fb_index, chunk) PE conv units emitted there
CONV_SCHED = {
    4: [(0, c) for c in range(0, 4)],
    5: [(0, c) for c in range(4, 8)],
    6: [(0, c) for c in range(8, 12)],
    7: [(0, c) for c in range(12, 15)],
    8: [(1, c) for c in range(0, 5)],
    9: [(1, c) for c in range(5, 10)],
}
CONV_TAIL_PE = [(1, c) for c in range(10, 15)]


def _build_program():
    import concourse.bass as bass
    import concourse.mybir as mybir
    from concourse.tile import TileContext

    dt = mybir.dt
    f32 = dt.float32
    f32r = dt.float32r
    Alu = mybir.AluOpType
    Act = mybir.ActivationFunctionType

    nc = bass.Bass()

    x_t = nc.dram_tensor("x", [3, NCH, 128, TPAD], f32, kind="ExternalInput")
    pr_t = nc.dram_tensor("pr", [16, NCH, 128], f32, kind="ExternalInput")
    out_t = nc.dram_tensor("out", [NCH, 128, T], f32, kind="ExternalOutput")

    ident_np = np.eye(128, dtype=np.float32)
    tl = np.arange(L, dtype=np.float32) + 0.5
    tl_np = np.tile(tl, (128, 1))
    lntl_np = np.tile(np.log(tl), (128, 1))
    ident_t = nc.inline_tensor(ident_np, "ident")
    tl_t = nc.inline_tensor(tl_np, "tlc")
    lntl_t = nc.inline_tensor(lntl_np, "lntlc")

    SERW = 16 + TPAD + 16
    ser_t = nc.dram_tensor("ser", [2, NCH, 128, SERW], f32, kind="Internal")

    with TileContext(nc) as tc:
        with (
            tc.tile_pool(name="pers", bufs=1) as pers,
            tc.tile_pool(name="blk", bufs=1) as blk,
            tc.tile_pool(name="pv", bufs=3) as pvp,
            tc.tile_pool(name="conv", bufs=2) as convp,
            tc.tile_pool(name="psum", bufs=2, space="PSUM") as psump,
        ):
            praw = pers.tile([128, 16 * NCH], f32, tag="praw", name="praw")
            NPAR = 21
            pd = pers.tile([128, NPAR * NCH], f32, tag="pd", name="pd")
            (P_W1P, P_C2P, P_EPSV, P_TBF, P_FE, P_LNKF, P_DDF, P_NDT,
             P_NETV, P_IVM, P_NCR, P_OMCV, P_CR, P_CV, P_CP, P_DPHI,
             P_VM, P_A1M, P_IB1, P_A2M, P_IB2) = range(NPAR)

            def pcol(j, c=None):
                if c is None:
                    return pd[:, j * NCH:(j + 1) * NCH]
                return pd[:, j * NCH + c:j * NCH + c + 1]

            ident = pers.tile([128, 128], f32, tag="ident", name="identt")
            tlt = pers.tile([128, L], f32, tag="tlt", name="tlt")
            lntlt = pers.tile([128, L], f32, tag="lntlt", name="lntlt")
            uhw = pers.tile([128, 2 * NCH * L], f32, tag="uhw", name="uhw")
            zeros16 = pers.tile([128, 16], f32, tag="z16", name="z16")
            cbias = pers.tile([128, 2], f32, tag="cbias", name="cbias")

            # scan states / scratch
            y2st = pers.tile([128, NCH], f32, tag="y2st", name="y2st")
            scp = pers.tile([128, 3 * NCH], f32, tag="scp", name="scp")  # y1|q|T4
            scv = pers.tile([128, 5 * NCH], f32, tag="scv", name="scv")  # z1|z2|z3|na|z4
            uss = pers.tile([128, NCH * (TB + 1)], f32, tag="uss", name="uss")
            vss = pers.tile([128, NCH * (TB + 1)], f32, tag="vss", name="vss")
            phs = pers.tile([128, NCH * (TB + 1)], f32, tag="phs", name="phs")

            raw = [blk.tile([128, 3 * NCH * TB], f32, tag=f"raw{i}", name=f"raw{i}")
                   for i in range(2)]
            sst = [blk.tile([128, 2 * NCH * TB], f32, tag=f"sst{i}", name=f"sst{i}")
                   for i in range(2)]
            vst = [blk.tile([128, 3 * NCH * TB], f32, tag=f"vst{i}", name=f"vst{i}")
                   for i in range(2)]
            ups = blk.tile([128, NCH * TB], f32, tag="ups", name="ups")
            vps = blk.tile([128, NCH * TB], f32, tag="vps", name="vps")
            paa = [blk.tile([128, TB], f32, tag=f"paa{i}", name=f"paa{i}") for i in range(2)]
            pab = [blk.tile([128, TB], f32, tag=f"pab{i}", name=f"pab{i}") for i in range(2)]
            pac = [blk.tile([128, TB], f32, tag=f"pac{i}", name=f"pac{i}") for i in range(2)]

            nc.sync.dma_start(ident[:], ident_t[:, :])
            nc.sync.dma_start(tlt[:], tl_t[:, :])
            nc.sync.dma_start(lntlt[:], lntl_t[:, :])
            nc.sync.dma_start(praw[:], pr_t.rearrange("j c p -> p (j c)"))
            nc.vector.memset(zeros16[:], 0.0)
            nc.vector.memset(cbias[:, 0:1], NZ)
            nc.vector.memset(cbias[:, 1:2], -1.0)

            def rawp(j):
                return praw[:, j * NCH:(j + 1) * NCH]

            ts = nc.vector.tensor_scalar
            tt = nc.vector.tensor_tensor
            stt = nc.vector.scalar_tensor_tensor
            pstt = nc.gpsimd.scalar_tensor_tensor
            pts = nc.gpsimd.tensor_scalar
            act = nc.scalar.activation

            # ---- derive params ----
            def ds(dst, j, lo, hi):
                ts(dst, rawp(j), float(hi - lo), float(lo), Alu.mult, Alu.add)

            ds(pcol(P_DDF), 0, 0.0, 40.0)
            ds(pcol(P_NDT), 1, -2.0, 3.0)                       # Tbm for now
            tt(pcol(P_NDT), pcol(P_DDF), pcol(P_NDT), Alu.mult)
            ts(pcol(P_NDT), pcol(P_NDT), -1.0, None, Alu.mult)  # -ddf*Tbm
            ds(pcol(P_W1P), 2, 1.0, 1.5)                        # 1 + wrf
            ds(pcol(P_TBF), 3, -5.0, 2.0)
            ds(pcol(P_LNKF), 4, 0.0, 5.0)                       # Kf for now
            act(pcol(P_LNKF), pcol(P_LNKF), Act.Ln)
            ts(pcol(P_LNKF), pcol(P_LNKF), -80.0, None, Alu.max)
            ds(pcol(P_FE), 5, 0.0, 1.0)
            ds(pcol(P_NETV), 6, 0.0, 1.0)                       # ETe for now
            ds(pcol(P_NCR), 7, 0.0, 1.0)                        # cr for now
            ds(pcol(P_C2P), 8, 1e-5, 0.02)
            ds(pcol(P_CV), 9, 0.0, 0.1)
            ds(pcol(P_CP), 10, 1e-5, 0.01)
            ds(pcol(P_VM), 11, 1e-3, 500.0)
            nc.vector.reciprocal(pcol(P_IVM), pcol(P_VM))
            tt(pcol(P_NETV), pcol(P_NETV), pcol(P_IVM), Alu.mult)
            ts(pcol(P_NETV), pcol(P_NETV), -1.0, None, Alu.mult)  # -ETe*ivm
            ts(pcol(P_EPSV), pcol(P_IVM), NZ, None, Alu.mult)
            ts(pcol(P_CR), pcol(P_NCR), 1.0, None, Alu.mult)      # cr
            ts(pcol(P_NCR), pcol(P_NCR), -1.0, None, Alu.mult)    # -cr
            ts(pcol(P_OMCV), pcol(P_CV), -1.0, 1.0, Alu.mult, Alu.add)  # 1-cv
            ts(pcol(P_DPHI), pcol(P_CP), -1.0, 1.0, Alu.mult, Alu.add)  # 1-cp
            ds(pcol(P_A1M), 12, 0.3, 20.0)
            ts(pcol(P_A1M), pcol(P_A1M), -1.0, None, Alu.add)
            ds(pcol(P_IB1), 13, 0.01, 5.0)
            nc.vector.reciprocal(pcol(P_IB1), pcol(P_IB1))
            ds(pcol(P_A2M), 14, 0.5, 13.0)
            ts(pcol(P_A2M), pcol(P_A2M), -1.0, None, Alu.add)
            ds(pcol(P_IB2), 15, 0.15, 1.5)
            nc.vector.reciprocal(pcol(P_IB2), pcol(P_IB2))

            # ---- state init ----
            nc.vector.memset(y2st[:], NZ)                        # Sg0 = NZ
            us4 = uss[:].rearrange("p (c t) -> p t c", c=NCH, t=TB + 1)
            vs4 = vss[:].rearrange("p (c t) -> p t c", c=NCH, t=TB + 1)
            ph4 = phs[:].rearrange("p (c t) -> p t c", c=NCH, t=TB + 1)
            nc.gpsimd.memset(us4[:, 0, :], 2 * NZ)               # U0 = Om0+Sg0
            ts(vs4[:, 0, :], pcol(P_EPSV), 0.0, None, Alu.add)   # V0 = eps/vm
            ts(ph4[:, 0, :], pcol(P_EPSV), 0.0, None, Alu.add)   # phi0

            # ---- UH weights + series zero prefixes (emitted after block 0) ----
            lgt = blk.tile([128, L], f32, tag="lgt", name="lgt")
            et = blk.tile([128, L], f32, tag="et", name="et")
            ssum = blk.tile([128, 1], f32, tag="ssum", name="ssum")

            def emit_uh_and_prefixes():
                for ui, amj in enumerate([P_A1M, P_A2M]):
                    for c in range(NCH):
                        am = pcol(amj, c)
                        ib = pcol(P_IB1, c) if ui == 0 else pcol(P_IB2, c)
                        ts(lgt[:], lntlt[:], am, None, Alu.mult)
                        stt(lgt[:], tlt[:], ib, lgt[:], Alu.mult, Alu.subtract)
                        act(et[:], lgt[:], Act.Exp, scale=-1.0)
                        nc.vector.tensor_reduce(ssum[:], et[:], mybir.AxisListType.X, Alu.add)
                        nc.vector.reciprocal(ssum[:], ssum[:])
                        ts(et[:], et[:], ssum[:], None, Alu.mult)
                        wdst = uhw[:, (ui * NCH + c) * L:(ui * NCH + c) * L + L]
                        ts(wdst, et[:], pcol(P_VM, c), None, Alu.mult)
                for c in range(NCH):
                    for si in range(2):
                        nc.sync.dma_start(ser_t[si, c, :, 0:16], zeros16[:])

            # ---- conv unit (PE, fp32r diag matmuls) ----
            def emit_conv_pe(fbi, c):
                f0, F = FBS[fbi]
                rhs = convp.tile([128, 2 * (F + 14)], f32, tag="rhs", name="rhs")
                nc.sync.dma_start(
                    rhs[:].rearrange("p (s t) -> p s t", s=2, t=F + 14),
                    ser_t[:, c, :, 2 + f0:2 + f0 + F + 14].rearrange("s p t -> p s t"))
                ps = psump.tile([128, F], f32, tag="ps", name="ps")
                for s in range(2):
                    for l in range(L):
                        dg = convp.tile([128, 128], f32, tag="dg", name="dg")
                        wcol = uhw[:, (s * NCH + c) * L + l:(s * NCH + c) * L + l + 1]
                        act(dg[:], ident[:], Act.Copy, scale=wcol)
                        nc.tensor.matmul(
                            ps[:, 0:F], dg[:].bitcast(f32r),
                            rhs[:, s * (F + 14) + 14 - l:s * (F + 14) + 14 - l + F].bitcast(f32r),
                            start=(s == 0 and l == 0), stop=(s == 1 and l == L - 1))
                ot = convp.tile([128, F], f32, tag="ot", name="ot")
                nc.scalar.copy(ot[:], ps[:, 0:F])
                nc.sync.dma_start(out_t[c, :, f0:f0 + F], ot[:])

            # ---- conv tail unit (DVE stt accumulate) ----
            def emit_conv_dve(fbi, c):
                f0, F = FBS[fbi]
                rhs = convp.tile([128, 2 * (F + 14)], f32, tag="rhs2", name="rhs2")
                nc.sync.dma_start(
                    rhs[:].rearrange("p (s t) -> p s t", s=2, t=F + 14),
                    ser_t[:, c, :, 2 + f0:2 + f0 + F + 14].rearrange("s p t -> p s t"))
                acc = convp.tile([128, F], f32, tag="acc", name="acc")
                first = True
                for s in range(2):
                    for l in range(L):
                        wcol = uhw[:, (s * NCH + c) * L + l:(s * NCH + c) * L + l + 1]
                        rsl = rhs[:, s * (F + 14) + 14 - l:s * (F + 14) + 14 - l + F]
                        if first:
                            ts(acc[:], rsl, wcol, None, Alu.mult)
                            first = False
                        else:
                            stt(acc[:], rsl, wcol, acc[:], Alu.mult, Alu.add)
                nc.sync.dma_start(out_t[c, :, f0:f0 + F], acc[:])

            # ---- phase A: derive streams for block b ----
            def emit_phase_a(b):
                pa = b % 2
                rw, ss, vs = raw[pa], sst[pa], vst[pa]
                for c in range(NCH):
                    Pc = rw[:, (c * 3 + 0) * TB:(c * 3 + 0) * TB + TB]
                    Tc = rw[:, (c * 3 + 1) * TB:(c * 3 + 1) * TB + TB]
                    Ec = rw[:, (c * 3 + 2) * TB:(c * 3 + 2) * TB + TB]
                    a_, b_, c_ = paa[c % 2], pab[c % 2], pac[c % 2]
                    PFd = ss[:, (0 * NCH + c) * TB:(0 * NCH + c) * TB + TB]
                    Ad = ss[:, (1 * NCH + c) * TB:(1 * NCH + c) * TB + TB]
                    NRd = vs[:, (2 * NCH + c) * TB:(2 * NCH + c) * TB + TB]
                    act(a_[:], Tc, Act.Relu, scale=-1.0, bias=pcol(P_TBF, c))
                    act(b_[:], a_[:], Act.Ln, bias=cbias[:, 0:1])
                    act(PFd, b_[:], Act.Exp, scale=pcol(P_FE, c), bias=pcol(P_LNKF, c))
                    act(a_[:], Tc, Act.Relu, scale=pcol(P_DDF, c), bias=pcol(P_NDT, c))
                    act(NRd, Ec, Act.Copy, scale=pcol(P_NETV, c))
                    stt(c_[:], Tc, 0.0, Pc, Alu.is_lt, Alu.mult)      # snow
                    tt(Ad, c_[:], a_[:], Alu.subtract)                # A = S - MP

            # ---- post-snow: avail -> Av, C streams for block b ----
            def emit_post_snow(b):
                pa = b % 2
                vs = vst[pa]
                for c in range(NCH):
                    c_ = pac[c % 2]
                    AVd = vs[:, (0 * NCH + c) * TB:(0 * NCH + c) * TB + TB]
                    Cd = vs[:, (1 * NCH + c) * TB:(1 * NCH + c) * TB + TB]
                    tt(c_[:], ups[:, c * TB:c * TB + TB],
                       uss[:, c * (TB + 1) + 1:c * (TB + 1) + 1 + TB], Alu.subtract)
                    act(AVd, c_[:], Act.Copy, scale=pcol(P_IVM, c))
                    act(Cd, AVd, Act.Identity, scale=pcol(P_NCR, c), bias=pcol(P_OMCV, c))

            # ---- post-V: series ho/qd2 for block bb, DMA to ser ----
            def emit_post_v(bb):
                t0 = bb * TB
                vsb = vst[bb % 2]
                for c in range(NCH):
                    base = c * (TB + 1)
                    VSh = vss[:, base:base + TB]
                    AVc = vsb[:, (0 * NCH + c) * TB:(0 * NCH + c) * TB + TB]
                    t_ovf = pvp.tile([128, TB], f32, tag="tovf", name="tovf")
                    t_v2 = pvp.tile([128, TB], f32, tag="tv2", name="tv2")
                    t_h1 = pvp.tile([128, TB], f32, tag="th1", name="th1")
                    t_cpp = pvp.tile([128, TB], f32, tag="tcpp", name="tcpp")
                    t_m1 = pvp.tile([128, TB], f32, tag="tm1", name="tm1")
                    hq = pvp.tile([128, 2 * TB], f32, tag="hq", name="hq")
                    act(t_ovf[:], vps[:, c * TB:c * TB + TB], Act.Relu, bias=cbias[:, 1:2])
                    act(t_v2[:], VSh, Act.Square)
                    act(t_h1[:], t_v2[:], Act.Copy, scale=pcol(P_C2P, c))
                    if bb > 0:
                        ts(phs[:, base:base + 1], phs[:, base + TB:base + TB + 1],
                           0.0, None, Alu.add)
                    nc.vector.tensor_tensor_scan(
                        phs[:, base + 1:base + 1 + TB],
                        pcol(P_DPHI, c).broadcast_to((128, TB)), t_h1[:],
                        phs[:, base:base + 1], Alu.mult, Alu.add)
                    act(t_cpp[:], phs[:, base:base + TB], Act.Copy, scale=pcol(P_CP, c))
                    tt(t_m1[:], VSh, AVc, Alu.mult)
                    stt(hq[:, 0:TB], t_m1[:], pcol(P_CR, c), t_ovf[:], Alu.mult, Alu.add)
                    stt(hq[:, TB:2 * TB], VSh, pcol(P_CV, c), t_cpp[:], Alu.mult, Alu.add)
                    nc.sync.dma_start(
                        ser_t[:, c, :, 16 + t0:16 + t0 + TB].rearrange("s p t -> p s t"),
                        hq[:].rearrange("p (s t) -> p s t", s=2, t=TB))

            # ---- raw input DMA for block b ----
            def emit_raw_dma(b):
                pa = b % 2
                t0 = b * TB
                for c in range(NCH):
                    nc.sync.dma_start(
                        raw[pa][:, c * 3 * TB:(c + 1) * 3 * TB].rearrange(
                            "p (ch t) -> p ch t", ch=3, t=TB),
                        x_t[:, c, :, t0:t0 + TB].rearrange("ch p t -> p ch t"))

            ups4 = ups[:].rearrange("p (c t) -> p t c", c=NCH, t=TB)
            vps4 = vps[:].rearrange("p (c t) -> p t c", c=NCH, t=TB)

            def S(i):
                return scp[:, i * NCH:(i + 1) * NCH]

            def Z(i):
                return scv[:, i * NCH:(i + 1) * NCH]

            emit_raw_dma(0)

            for b in range(NB + 1):
                if b < NB:
                    emit_phase_a(b)
                    if b + 1 < NB:
                        emit_raw_dma(b + 1)
                    if b >= 1:
                        pts(us4[:, 0, :], us4[:, TB, :], 0.0, None, Alu.add)
                if b >= 2:
                    ts(vs4[:, 0, :], vs4[:, TB, :], 0.0, None, Alu.add)

                sm = (sst[b % 2][:].rearrange("p (s c t) -> p t (s c)", s=2, c=NCH, t=TB)
                      if b < NB else None)
                pv = (raw[b % 2][:].rearrange("p (c s t) -> p t s c", c=NCH, s=3, t=TB)
                      if b < NB else None)
                vm4 = (vst[(b - 1) % 2][:].rearrange("p (s c t) -> p t (s c)", s=3, c=NCH, t=TB)
                       if b >= 1 else None)

                for t in range(TB):
                    if b < NB:
                        PFt = sm[:, t, 0:NCH]
                        At = sm[:, t, NCH:2 * NCH]
                        Pt = pv[:, t, 0, :]
                        Ut = us4[:, t, :]
                        pstt(S(0), y2st[:], 0.0, PFt, Alu.max, Alu.add)       # y1
                        pstt(S(1), S(0), 0.0, Ut, Alu.max, Alu.min)           # q
                        pstt(y2st[:], S(1), 1.0, At, Alu.mult, Alu.add)       # y2'
                        pstt(S(2), y2st[:], 0.0, pcol(P_W1P), Alu.max, Alu.mult)  # T4
                        pstt(ups4[:, t, :], Ut, 1.0, Pt, Alu.mult, Alu.add)   # UP
                        pstt(us4[:, t + 1, :], ups4[:, t, :], 0.0, S(2),
                             Alu.max, Alu.min)                                # U'
                    if b >= 1:
                        AVt = vm4[:, t, 0:NCH]
                        Ct = vm4[:, t, NCH:2 * NCH]
                        NRt = vm4[:, t, 2 * NCH:3 * NCH]
                        Vt = vs4[:, t, :]
                        tt(Z(0), pcol(P_C2P), Vt, Alu.mult)                   # z1
                        tt(Z(1), Ct, Z(0), Alu.subtract)                      # z2
                        tt(Z(2), Vt, Z(1), Alu.mult)                          # z3
                        stt(Z(3), Vt, -1.0, NRt, Alu.mult, Alu.max)           # na
                        tt(Z(4), Z(2), AVt, Alu.add)                          # z4
                        tt(vps4[:, t, :], Z(4), Z(3), Alu.add)                # Vp
                        stt(vs4[:, t + 1, :], vps4[:, t, :], 1.0, pcol(P_EPSV),
                            Alu.min, Alu.max)                                 # V'

                if b < NB:
                    emit_post_snow(b)
                if b >= 1:
                    emit_post_v(b - 1)
                if b == 0:
                    emit_uh_and_prefixes()
                for fbi, c in CONV_SCHED.get(b, []):
                    emit_conv_pe(fbi, c)

            for fbi, c in CONV_TAIL_PE:
                emit_conv_pe(fbi, c)
            for c in range(NCH):
                emit_conv_dve(2, c)

    _strip_same_engine_waits(nc)
    _split_multi_waits(nc)
    return nc


def _strip_same_engine_waits(nc):
    """Drop semaphore waits that only order an engine against itself.

    Engines execute their instruction queue in order, so a wait on a sem
    whose every updater is a non-DMA instruction on the same engine is
    redundant ordering-wise; TimelineSim charges ~95ns per such wait
    (producer side-effect drain + sem propagation).  Cross-engine waits and
    DMA-completion waits (sems updated by DMA-class instructions, which fire
    at transfer completion, not instruction retirement) are kept.
    """
    upd_engines = {}
    dma_like = ("DMA", "TriggerDma")

    def sem_key(x):
        return (x.sync_type, x.id)

    for f in nc.m.functions:
        for bb in f.blocks:
            for ins in bb.instructions:
                si = ins.sync_info
                if si is None or not si.on_update:
                    continue
                is_dma = any(s in ins.opcode for s in dma_like)
                for u in si.on_update:
                    upd_engines.setdefault(sem_key(u), set()).add(
                        "DMA" if is_dma else ins.engine)

    stripped = 0
    for f in nc.m.functions:
        for bb in f.blocks:
            for ins in bb.instructions:
                si = ins.sync_info
                if si is None or not si.on_wait:
                    continue
                keep = []
                for w in si.on_wait:
                    engs = upd_engines.get(sem_key(w), {"?"})
                    if engs == {ins.engine}:
                        stripped += 1
                    else:
                        keep.append(w)
                if len(keep) != len(si.on_wait):
                    si.on_wait = keep


def _split_multi_waits(nc):
    """This container's walrus codegen accepts at most ONE sync wait per
    instruction; Tile emits several.  Hoist the excess onto same-engine
    NoOp carriers inserted immediately before."""
    from bass_rust import InstNoOp, SyncInfo

    cnt = 0
    for f in nc.m.functions:
        for bb in f.blocks:
            out = []
            changed = False
            for ins in bb.instructions:
                si = ins.sync_info
                w = list(si.on_wait) if si is not None and si.on_wait else []
                if len(w) > 1:
                    for extra in w[:-1]:
                        cnt += 1
                        nop = InstNoOp(name=f"WQ-{cnt}", engine=ins.engine)
                        nop.sync_info = SyncInfo(on_wait=[extra], on_update=[])
                        out.append(nop)
                    si.on_wait = [w[-1]]
                    changed = True
                out.append(ins)
            if changed:
                bb.instructions = out


def _get_program():
    if "nc" not in _CACHE:
        _CACHE["nc"] = _build_program()
    return _CACHE["nc"]


def kernel(x_phy: np.ndarray, params: np.ndarray) -> np.ndarray:
    from concourse.bass_utils import run_bass_kernel_spmd

    nc = _get_program()

    x_phy = np.ascontiguousarray(x_phy, dtype=np.float32)
    params = np.ascontiguousarray(params, dtype=np.float32)

    GPAD = NCORES * GC
    xp = np.zeros((TPAD, GPAD, 3), np.float32)
    xp[:T, :G] = x_phy
    pp = np.full((GPAD, 16), 0.5, np.float32)
    pp[:G] = params

    in_maps = []
    for k in range(NCORES):
        g0 = k * GC
        xk = np.ascontiguousarray(
            xp[:, g0:g0 + GC].transpose(2, 1, 0).reshape(3, NCH, 128, TPAD))
        pk = np.ascontiguousarray(
            pp[g0:g0 + GC].reshape(NCH, 128, 16).transpose(2, 0, 1))
        in_maps.append({"x": xk, "pr": pk})

    res = run_bass_kernel_spmd(nc, in_maps, core_ids=list(range(NCORES)))

    out = np.empty((T, G), np.float32)
    for k in range(NCORES):
        o = res.results[k]["out"]            # [NCH,128,T]
        g0 = k * GC
        hi = min(g0 + GC, G)
        flat = o.transpose(2, 0, 1).reshape(T, GC)
        out[:, g0:hi] = flat[:, :hi - g0]
    return out


# revision 8
# speedup vs baseline: 1.8417x; 1.1153x over previous
"""Exphydro (snow + 2-bucket soil + gamma-UH routing) Trainium2 Bass kernel.

Contract: kernel(x_phy [1095,15000,3] f32, params [15000,16] f32) -> [1095,15000] f32.
Shards the grid dim across 8 NeuronCores (1875 -> padded 1920 per core).

v2 architecture (vs v1's 24 serial ops/step):
  - snow recurrence rewritten in (pre-relu Sg, U=Om+Sg) coordinates: 6 Pool
    stt ops/step; avail = U + P - U' recovered in batch per block.
  - vadose recurrence grouped as V' = clip(V(C - c2p V) + Av - min(RP,V)):
    7 DVE ops/step, running one block behind the snow scan.
  - all series terms (ho, h1, qd2=h2+cp*phi, phi via tensor_tensor_scan)
    computed in batch [128, 128] ops, mostly on ACT with per-chunk [128,1]
    scale/bias vectors.
  - gamma-UH conv: 2 series x 15 taps as diag matmuls in fp32r (1 cyc/row at
    F>=256) on the PE, plus a 71-wide tail block on DVE.
"""
import numpy as np

T = 1095
TB = 128
NB = 9
TPAD = TB * NB           # 1152
G = 15000
NCORES = 8
GC = 1920                # padded grid per core
NCH = 15                 # chunks of 128 per core
L = 15                   # UH length
NZ = 1e-5

_CACHE = {}

# conv F-blocks: two PE groups of 512, one DVE tail of 71
FBS = [(0, 512), (512, 512), (1024, 71)]
# iteration -> list of (fb_index, chunk) PE conv units emitted there
CONV_SCHED = {
    4: [(0, c) for c in range(0, 4)],
    5: [(0, c) for c in range(4, 8)],
    6: [(0, c) for c in range(8, 12)],
    7: [(0, c) for c in range(12, 15)],
    8: [(1, c) for c in range(0, 5)],
    9: [(1, c) for c in range(5, 10)],
}
CONV_TAIL_PE = [(1, c) for c in range(10, 15)]


def _build_program():
    import concourse.bass as bass
    import concourse.mybir as mybir
    from concourse.tile import TileContext

    dt = mybir.dt
    f32 = dt.float32
    f32r = dt.float32r
    Alu = mybir.AluOpType
    Act = mybir.ActivationFunctionType

    nc = bass.Bass()

    x_t = nc.dram_tensor("x", [3, NCH, 128, TPAD], f32, kind="ExternalInput")
    pr_t = nc.dram_tensor("pr", [16, NCH, 128], f32, kind="ExternalInput")
    out_t = nc.dram_tensor("out", [NCH, 128, T], f32, kind="ExternalOutput")

    ident_np = np.eye(128, dtype=np.float32)
    tl = np.arange(L, dtype=np.float32) + 0.5
    tl_np = np.tile(tl, (128, 1))
    lntl_np = np.tile(np.log(tl), (128, 1))
    ident_t = nc.inline_tensor(ident_np, "ident")
    tl_t = nc.inline_tensor(tl_np, "tlc")
    lntl_t = nc.inline_tensor(lntl_np, "lntlc")

    SERW = 16 + TPAD + 16
    ser_t = nc.dram_tensor("ser", [2, NCH, 128, SERW], f32, kind="Internal")

    with TileContext(nc) as tc:
        with (
            tc.tile_pool(name="pers", bufs=1) as pers,
            tc.tile_pool(name="blk", bufs=1) as blk,
            tc.tile_pool(name="pv", bufs=3) as pvp,
            tc.tile_pool(name="conv", bufs=2) as convp,
            tc.tile_pool(name="psum", bufs=2, space="PSUM") as psump,
        ):
            praw = pers.tile([128, 16 * NCH], f32, tag="praw", name="praw")
            NPAR = 23
            pd = pers.tile([128, NPAR * NCH], f32, tag="pd", name="pd")
            (P_W1P, P_C2P, P_EPSV, P_TBF, P_FE, P_LNKF, P_DDF, P_NDT,
             P_NETV, P_IVM, P_NCR, P_OMCV, P_CR, P_CV, P_CP, P_DPHI,
             P_VM, P_A1M, P_IB1, P_A2M, P_IB2, P_PC2P, P_PSI0) = range(NPAR)

            def pcol(j, c=None):
                if c is None:
                    return pd[:, j * NCH:(j + 1) * NCH]
                return pd[:, j * NCH + c:j * NCH + c + 1]

            ident = pers.tile([128, 128], f32, tag="ident", name="identt")
            tlt = pers.tile([128, L], f32, tag="tlt", name="tlt")
            lntlt = pers.tile([128, L], f32, tag="lntlt", name="lntlt")
            uhw = pers.tile([128, 2 * NCH * L], f32, tag="uhw", name="uhw")
            zeros16 = pers.tile([128, 16], f32, tag="z16", name="z16")
            cbias = pers.tile([128, 2], f32, tag="cbias", name="cbias")

            # scan states / scratch
            y2st = pers.tile([128, NCH], f32, tag="y2st", name="y2st")
            scp = pers.tile([128, 3 * NCH], f32, tag="scp", name="scp")  # y1|q|T4
            scv = pers.tile([128, 5 * NCH], f32, tag="scv", name="scv")  # z1|z2|z3|na|z4
            uss = [pers.tile([128, NCH * (TB + 1)], f32, tag=f"uss{i}", name=f"uss{i}")
                   for i in range(2)]
            vss = pers.tile([128, NCH * (TB + 1)], f32, tag="vss", name="vss")
            phs = pers.tile([128, NCH * (TB + 1)], f32, tag="phs", name="phs")

            raw = [blk.tile([128, 3 * NCH * TB], f32, tag=f"raw{i}", name=f"raw{i}")
                   for i in range(2)]
            sst = [blk.tile([128, 2 * NCH * TB], f32, tag=f"sst{i}", name=f"sst{i}")
                   for i in range(2)]
            avst = [blk.tile([128, 2 * NCH * TB], f32, tag=f"avst{i}", name=f"avst{i}")
                    for i in range(2)]
            nrst = [blk.tile([128, NCH * TB], f32, tag=f"nrst{i}", name=f"nrst{i}")
                    for i in range(2)]
            ups = [blk.tile([128, NCH * TB], f32, tag=f"ups{i}", name=f"ups{i}")
                   for i in range(2)]
            vps = blk.tile([128, NCH * TB], f32, tag="vps", name="vps")
            paa = [blk.tile([128, TB], f32, tag=f"paa{i}", name=f"paa{i}") for i in range(2)]
            pab = [blk.tile([128, TB], f32, tag=f"pab{i}", name=f"pab{i}") for i in range(2)]
            pac = [blk.tile([128, TB], f32, tag=f"pac{i}", name=f"pac{i}") for i in range(2)]

            nc.sync.dma_start(ident[:], ident_t[:, :])
            nc.sync.dma_start(tlt[:], tl_t[:, :])
            nc.sync.dma_start(lntlt[:], lntl_t[:, :])
            nc.sync.dma_start(praw[:], pr_t.rearrange("j c p -> p (j c)"))
            nc.vector.memset(zeros16[:], 0.0)
            nc.vector.memset(cbias[:, 0:1], NZ)
            nc.vector.memset(cbias[:, 1:2], -1.0)

            def rawp(j):
                return praw[:, j * NCH:(j + 1) * NCH]

            ts = nc.vector.tensor_scalar
            tt = nc.vector.tensor_tensor
            stt = nc.vector.scalar_tensor_tensor
            pstt = nc.gpsimd.scalar_tensor_tensor
            pts = nc.gpsimd.tensor_scalar
            act = nc.scalar.activation

            # ---- derive params ----
            def ds(dst, j, lo, hi):
                ts(dst, rawp(j), float(hi - lo), float(lo), Alu.mult, Alu.add)

            ds(pcol(P_DDF), 0, 0.0, 40.0)
            ds(pcol(P_NDT), 1, -2.0, 3.0)                       # Tbm for now
            tt(pcol(P_NDT), pcol(P_DDF), pcol(P_NDT), Alu.mult)
            ts(pcol(P_NDT), pcol(P_NDT), -1.0, None, Alu.mult)  # -ddf*Tbm
            ds(pcol(P_W1P), 2, 1.0, 1.5)                        # 1 + wrf
            ds(pcol(P_TBF), 3, -5.0, 2.0)
            ds(pcol(P_LNKF), 4, 0.0, 5.0)                       # Kf for now
            act(pcol(P_LNKF), pcol(P_LNKF), Act.Ln)
            ts(pcol(P_LNKF), pcol(P_LNKF), -80.0, None, Alu.max)
            ds(pcol(P_FE), 5, 0.0, 1.0)
            ds(pcol(P_NETV), 6, 0.0, 1.0)                       # ETe for now
            ds(pcol(P_NCR), 7, 0.0, 1.0)                        # cr for now
            ds(pcol(P_C2P), 8, 1e-5, 0.02)
            ds(pcol(P_CV), 9, 0.0, 0.1)
            ds(pcol(P_CP), 10, 1e-5, 0.01)
            ds(pcol(P_VM), 11, 1e-3, 500.0)
            nc.vector.reciprocal(pcol(P_IVM), pcol(P_VM))
            tt(pcol(P_NETV), pcol(P_NETV), pcol(P_IVM), Alu.mult)
            ts(pcol(P_NETV), pcol(P_NETV), -1.0, None, Alu.mult)  # -ETe*ivm
            ts(pcol(P_EPSV), pcol(P_IVM), NZ, None, Alu.mult)
            ts(pcol(P_CR), pcol(P_NCR), 1.0, None, Alu.mult)      # cr
            ts(pcol(P_NCR), pcol(P_NCR), -1.0, None, Alu.mult)    # -cr
            ts(pcol(P_OMCV), pcol(P_CV), -1.0, 1.0, Alu.mult, Alu.add)  # 1-cv
            ts(pcol(P_DPHI), pcol(P_CP), -1.0, 1.0, Alu.mult, Alu.add)  # 1-cp
            tt(pcol(P_PC2P), pcol(P_CP), pcol(P_C2P), Alu.mult)   # cp*c2p
            tt(pcol(P_PSI0), pcol(P_CP), pcol(P_EPSV), Alu.mult)  # psi init
            ds(pcol(P_A1M), 12, 0.3, 20.0)
            ts(pcol(P_A1M), pcol(P_A1M), -1.0, None, Alu.add)
            ds(pcol(P_IB1), 13, 0.01, 5.0)
            nc.vector.reciprocal(pcol(P_IB1), pcol(P_IB1))
            ds(pcol(P_A2M), 14, 0.5, 13.0)
            ts(pcol(P_A2M), pcol(P_A2M), -1.0, None, Alu.add)
            ds(pcol(P_IB2), 15, 0.15, 1.5)
            nc.vector.reciprocal(pcol(P_IB2), pcol(P_IB2))

            # ---- state init ----
            nc.vector.memset(y2st[:], NZ)                        # Sg0 = NZ
            us4 = [u[:].rearrange("p (c t) -> p t c", c=NCH, t=TB + 1) for u in uss]
            vs4 = vss[:].rearrange("p (c t) -> p t c", c=NCH, t=TB + 1)
            ph4 = phs[:].rearrange("p (c t) -> p t c", c=NCH, t=TB + 1)
            nc.gpsimd.memset(us4[0][:, 0, :], 2 * NZ)            # U0 = Om0+Sg0
            ts(vs4[:, 0, :], pcol(P_EPSV), 0.0, None, Alu.add)   # V0 = eps/vm
            ts(ph4[:, 0, :], pcol(P_PSI0), 0.0, None, Alu.add)   # psi0 = cp*eps/vm

            # ---- UH weights + series zero prefixes (emitted after block 0) ----
            lgt = blk.tile([128, L], f32, tag="lgt", name="lgt")
            et = blk.tile([128, L], f32, tag="et", name="et")
            ssum = blk.tile([128, 1], f32, tag="ssum", name="ssum")

            def emit_uh_and_prefixes():
                for ui, amj in enumerate([P_A1M, P_A2M]):
                    for c in range(NCH):
                        am = pcol(amj, c)
                        ib = pcol(P_IB1, c) if ui == 0 else pcol(P_IB2, c)
                        ts(lgt[:], lntlt[:], am, None, Alu.mult)
                        stt(lgt[:], tlt[:], ib, lgt[:], Alu.mult, Alu.subtract)
                        act(et[:], lgt[:], Act.Exp, scale=-1.0)
                        nc.vector.tensor_reduce(ssum[:], et[:], mybir.AxisListType.X, Alu.add)
                        nc.vector.reciprocal(ssum[:], ssum[:])
                        ts(et[:], et[:], ssum[:], None, Alu.mult)
                        wdst = uhw[:, (ui * NCH + c) * L:(ui * NCH + c) * L + L]
                        ts(wdst, et[:], pcol(P_VM, c), None, Alu.mult)
                for c in range(NCH):
                    for si in range(2):
                        nc.sync.dma_start(ser_t[si, c, :, 0:16], zeros16[:])

            # ---- conv unit (PE, fp32r diag matmuls) ----
            def emit_conv_pe(fbi, c):
                f0, F = FBS[fbi]
                rhs = convp.tile([128, 2 * (F + 14)], f32, tag="rhs", name="rhs")
                nc.sync.dma_start(
                    rhs[:].rearrange("p (s t) -> p s t", s=2, t=F + 14),
                    ser_t[:, c, :, 2 + f0:2 + f0 + F + 14].rearrange("s p t -> p s t"))
                ps = psump.tile([128, F], f32, tag="ps", name="ps")
                for s in range(2):
                    for l in range(L):
                        dg = convp.tile([128, 128], f32, tag="dg", name="dg")
                        wcol = uhw[:, (s * NCH + c) * L + l:(s * NCH + c) * L + l + 1]
                        act(dg[:], ident[:], Act.Copy, scale=wcol)
                        nc.tensor.matmul(
                            ps[:, 0:F], dg[:].bitcast(f32r),
                            rhs[:, s * (F + 14) + 14 - l:s * (F + 14) + 14 - l + F].bitcast(f32r),
                            start=(s == 0 and l == 0), stop=(s == 1 and l == L - 1))
                ot = convp.tile([128, F], f32, tag="ot", name="ot")
                nc.scalar.copy(ot[:], ps[:, 0:F])
                nc.sync.dma_start(out_t[c, :, f0:f0 + F], ot[:])

            # ---- conv tail unit (DVE stt accumulate) ----
            def emit_conv_dve(fbi, c):
                f0, F = FBS[fbi]
                rhs = convp.tile([128, 2 * (F + 14)], f32, tag="rhs2", name="rhs2")
                nc.sync.dma_start(
                    rhs[:].rearrange("p (s t) -> p s t", s=2, t=F + 14),
                    ser_t[:, c, :, 2 + f0:2 + f0 + F + 14].rearrange("s p t -> p s t"))
                acc = convp.tile([128, F], f32, tag="acc", name="acc")
                first = True
                for s in range(2):
                    for l in range(L):
                        wcol = uhw[:, (s * NCH + c) * L + l:(s * NCH + c) * L + l + 1]
                        rsl = rhs[:, s * (F + 14) + 14 - l:s * (F + 14) + 14 - l + F]
                        if first:
                            ts(acc[:], rsl, wcol, None, Alu.mult)
                            first = False
                        else:
                            stt(acc[:], rsl, wcol, acc[:], Alu.mult, Alu.add)
                nc.sync.dma_start(out_t[c, :, f0:f0 + F], acc[:])

            # ---- phase A: derive streams for block b ----
            def emit_phase_a(b):
                pa = b % 2
                rw, ss, nr = raw[pa], sst[pa], nrst[pa]
                for c in range(NCH):
                    Pc = rw[:, (c * 3 + 0) * TB:(c * 3 + 0) * TB + TB]
                    Tc = rw[:, (c * 3 + 1) * TB:(c * 3 + 1) * TB + TB]
                    Ec = rw[:, (c * 3 + 2) * TB:(c * 3 + 2) * TB + TB]
                    a_, b_, c_ = paa[c % 2], pab[c % 2], pac[c % 2]
                    PFd = ss[:, (0 * NCH + c) * TB:(0 * NCH + c) * TB + TB]
                    Ad = ss[:, (1 * NCH + c) * TB:(1 * NCH + c) * TB + TB]
                    NRd = nr[:, c * TB:c * TB + TB]
                    act(a_[:], Tc, Act.Relu, scale=-1.0, bias=pcol(P_TBF, c))
                    act(b_[:], a_[:], Act.Ln, bias=cbias[:, 0:1])
                    act(PFd, b_[:], Act.Exp, scale=pcol(P_FE, c), bias=pcol(P_LNKF, c))
                    act(a_[:], Tc, Act.Relu, scale=pcol(P_DDF, c), bias=pcol(P_NDT, c))
                    act(NRd, Ec, Act.Copy, scale=pcol(P_NETV, c))
                    stt(c_[:], Tc, 0.0, Pc, Alu.is_lt, Alu.mult)      # snow
                    tt(Ad, c_[:], a_[:], Alu.subtract)                # A = S - MP

            # ---- post-snow: avail -> Av, C streams for block b ----
            def emit_post_snow(b):
                pa = b % 2
                vs = avst[pa]
                for c in range(NCH):
                    c_ = pac[c % 2]
                    AVd = vs[:, (0 * NCH + c) * TB:(0 * NCH + c) * TB + TB]
                    Cd = vs[:, (1 * NCH + c) * TB:(1 * NCH + c) * TB + TB]
                    tt(c_[:], ups[pa][:, c * TB:c * TB + TB],
                       uss[pa][:, c * (TB + 1) + 1:c * (TB + 1) + 1 + TB], Alu.subtract)
                    act(AVd, c_[:], Act.Copy, scale=pcol(P_IVM, c))
                    act(Cd, AVd, Act.Identity, scale=pcol(P_NCR, c), bias=pcol(P_OMCV, c))

            # ---- post-V: series ho/qd2 for block bb, DMA to ser ----
            def emit_post_v(bb):
                t0 = bb * TB
                vsb = avst[bb % 2]
                for c in range(NCH):
                    base = c * (TB + 1)
                    VSh = vss[:, base:base + TB]
                    AVc = vsb[:, (0 * NCH + c) * TB:(0 * NCH + c) * TB + TB]
                    t_ovf = pvp.tile([128, TB], f32, tag="tovf", name="tovf")
                    t_v2 = pvp.tile([128, TB], f32, tag="tv2", name="tv2")
                    t_h1 = pvp.tile([128, TB], f32, tag="th1", name="th1")
                    t_m1 = pvp.tile([128, TB], f32, tag="tm1", name="tm1")
                    hq = pvp.tile([128, 2 * TB], f32, tag="hq", name="hq")
                    act(t_ovf[:], vps[:, c * TB:c * TB + TB], Act.Relu, bias=cbias[:, 1:2])
                    act(t_v2[:], VSh, Act.Square)
                    act(t_h1[:], t_v2[:], Act.Copy, scale=pcol(P_PC2P, c))
                    if bb > 0:
                        ts(phs[:, base:base + 1], phs[:, base + TB:base + TB + 1],
                           0.0, None, Alu.add)
                    nc.vector.tensor_tensor_scan(
                        phs[:, base + 1:base + 1 + TB],
                        pcol(P_DPHI, c).broadcast_to((128, TB)), t_h1[:],
                        phs[:, base:base + 1], Alu.mult, Alu.add)
                    tt(t_m1[:], VSh, AVc, Alu.mult)
                    stt(hq[:, 0:TB], t_m1[:], pcol(P_CR, c), t_ovf[:], Alu.mult, Alu.add)
                    stt(hq[:, TB:2 * TB], VSh, pcol(P_CV, c),
                        phs[:, base:base + TB], Alu.mult, Alu.add)
                    nc.sync.dma_start(
                        ser_t[:, c, :, 16 + t0:16 + t0 + TB].rearrange("s p t -> p s t"),
                        hq[:].rearrange("p (s t) -> p s t", s=2, t=TB))

            # ---- raw input DMA for block b ----
            def emit_raw_dma(b):
                pa = b % 2
                t0 = b * TB
                for c in range(NCH):
                    nc.sync.dma_start(
                        raw[pa][:, c * 3 * TB:(c + 1) * 3 * TB].rearrange(
                            "p (ch t) -> p ch t", ch=3, t=TB),
                        x_t[:, c, :, t0:t0 + TB].rearrange("ch p t -> p ch t"))

            ups4 = [u[:].rearrange("p (c t) -> p t c", c=NCH, t=TB) for u in ups]
            vps4 = vps[:].rearrange("p (c t) -> p t c", c=NCH, t=TB)

            def S(i):
                return scp[:, i * NCH:(i + 1) * NCH]

            def Z(i):
                return scv[:, i * NCH:(i + 1) * NCH]

            emit_raw_dma(0)
            emit_phase_a(0)

            for b in range(NB + 1):
                if b + 1 < NB:
                    emit_raw_dma(b + 1)
                    emit_phase_a(b + 1)
                if b >= 1 and b < NB:
                    pts(us4[b % 2][:, 0, :], us4[(b - 1) % 2][:, TB, :],
                        0.0, None, Alu.add)
                if b >= 2:
                    ts(vs4[:, 0, :], vs4[:, TB, :], 0.0, None, Alu.add)

                sm = (sst[b % 2][:].rearrange("p (s c t) -> p t (s c)", s=2, c=NCH, t=TB)
                      if b < NB else None)
                pv = (raw[b % 2][:].rearrange("p (c s t) -> p t s c", c=NCH, s=3, t=TB)
                      if b < NB else None)
                vm4 = (avst[(b - 1) % 2][:].rearrange("p (s c t) -> p t (s c)", s=2, c=NCH, t=TB)
                       if b >= 1 else None)
                nr4 = (nrst[(b - 1) % 2][:].rearrange("p (c t) -> p t c", c=NCH, t=TB)
                       if b >= 1 else None)

                for t in range(TB):
                    if b < NB:
                        PFt = sm[:, t, 0:NCH]
                        At = sm[:, t, NCH:2 * NCH]
                        Pt = pv[:, t, 0, :]
                        Ut = us4[b % 2][:, t, :]
                        pstt(S(0), y2st[:], 0.0, PFt, Alu.max, Alu.add)       # y1
                        pstt(S(1), S(0), 0.0, Ut, Alu.max, Alu.min)           # q
                        pstt(y2st[:], S(1), 1.0, At, Alu.mult, Alu.add)       # y2'
                        pstt(S(2), y2st[:], 0.0, pcol(P_W1P), Alu.max, Alu.mult)  # T4
                        pstt(ups4[b % 2][:, t, :], Ut, 1.0, Pt, Alu.mult, Alu.add)   # UP
                        pstt(us4[b % 2][:, t + 1, :], ups4[b % 2][:, t, :], 0.0, S(2),
                             Alu.max, Alu.min)                                # U'
                    if b >= 1:
                        AVt = vm4[:, t, 0:NCH]
                        Ct = vm4[:, t, NCH:2 * NCH]
                        NRt = nr4[:, t, :]
                        Vt = vs4[:, t, :]
                        tt(Z(0), pcol(P_C2P), Vt, Alu.mult)                   # z1
                        tt(Z(1), Ct, Z(0), Alu.subtract)                      # z2
                        tt(Z(2), Vt, Z(1), Alu.mult)                          # z3
                        stt(Z(3), Vt, -1.0, NRt, Alu.mult, Alu.max)           # na
                        tt(Z(4), Z(2), AVt, Alu.add)                          # z4
                        tt(vps4[:, t, :], Z(4), Z(3), Alu.add)                # Vp
                        stt(vs4[:, t + 1, :], vps4[:, t, :], 1.0, pcol(P_EPSV),
                            Alu.min, Alu.max)                                 # V'

                if b < NB:
                    with tc.high_priority():
                        emit_post_snow(b)
                if b >= 1:
                    emit_post_v(b - 1)
                if b == 0:
                    emit_uh_and_prefixes()
                for fbi, c in CONV_SCHED.get(b, []):
                    emit_conv_pe(fbi, c)

            for fbi, c in CONV_TAIL_PE:
                emit_conv_pe(fbi, c)
            for c in range(NCH):
                emit_conv_dve(2, c)

    _strip_same_engine_waits(nc)
    _split_multi_waits(nc)
    return nc


def _strip_same_engine_waits(nc):
    """Drop semaphore waits that only order an engine against itself.

    Engines execute their instruction queue in order, so a wait on a sem
    whose every updater is a non-DMA instruction on the same engine is
    redundant ordering-wise; TimelineSim charges ~95ns per such wait
    (producer side-effect drain + sem propagation).  Cross-engine waits and
    DMA-completion waits (sems updated by DMA-class instructions, which fire
    at transfer completion, not instruction retirement) are kept.
    """
    upd_engines = {}
    dma_like = ("DMA", "TriggerDma")

    def sem_key(x):
        return (x.sync_type, x.id)

    for f in nc.m.functions:
        for bb in f.blocks:
            for ins in bb.instructions:
                si = ins.sync_info
                if si is None or not si.on_update:
                    continue
                is_dma = any(s in ins.opcode for s in dma_like)
                for u in si.on_update:
                    upd_engines.setdefault(sem_key(u), set()).add(
                        "DMA" if is_dma else ins.engine)

    stripped = 0
    for f in nc.m.functions:
        for bb in f.blocks:
            for ins in bb.instructions:
                si = ins.sync_info
                if si is None or not si.on_wait:
                    continue
                keep = []
                for w in si.on_wait:
                    engs = upd_engines.get(sem_key(w), {"?"})
                    if engs == {ins.engine}:
                        stripped += 1
                    else:
                        keep.append(w)
                if len(keep) != len(si.on_wait):
                    si.on_wait = keep


def _split_multi_waits(nc):
    """This container's walrus codegen accepts at most ONE sync wait per
    instruction; Tile emits several.  Hoist the excess onto same-engine
    NoOp carriers inserted immediately before."""
    from bass_rust import InstNoOp, SyncInfo

    cnt = 0
    for f in nc.m.functions:
        for bb in f.blocks:
            out = []
            changed = False
            for ins in bb.instructions:
                si = ins.sync_info
                w = list(si.on_wait) if si is not None and si.on_wait else []
                if len(w) > 1:
                    for extra in w[:-1]:
                        cnt += 1
                        nop = InstNoOp(name=f"WQ-{cnt}", engine=ins.engine)
                        nop.sync_info = SyncInfo(on_wait=[extra], on_update=[])
                        out.append(nop)
                    si.on_wait = [w[-1]]
                    changed = True
                out.append(ins)
            if changed:
                bb.instructions = out


def _get_program():
    if "nc" not in _CACHE:
        _CACHE["nc"] = _build_program()
    return _CACHE["nc"]


def kernel(x_phy: np.ndarray, params: np.ndarray) -> np.ndarray:
    from concourse.bass_utils import run_bass_kernel_spmd

    nc = _get_program()

    x_phy = np.ascontiguousarray(x_phy, dtype=np.float32)
    params = np.ascontiguousarray(params, dtype=np.float32)

    GPAD = NCORES * GC
    xp = np.zeros((TPAD, GPAD, 3), np.float32)
    xp[:T, :G] = x_phy
    pp = np.full((GPAD, 16), 0.5, np.float32)
    pp[:G] = params

    in_maps = []
    for k in range(NCORES):
        g0 = k * GC
        xk = np.ascontiguousarray(
            xp[:, g0:g0 + GC].transpose(2, 1, 0).reshape(3, NCH, 128, TPAD))
        pk = np.ascontiguousarray(
            pp[g0:g0 + GC].reshape(NCH, 128, 16).transpose(2, 0, 1))
        in_maps.append({"x": xk, "pr": pk})

    res = run_bass_kernel_spmd(nc, in_maps, core_ids=list(range(NCORES)))

    out = np.empty((T, G), np.float32)
    for k in range(NCORES):
        o = res.results[k]["out"]            # [NCH,128,T]
        g0 = k * GC
        hi = min(g0 + GC, G)
        flat = o.transpose(2, 0, 1).reshape(T, GC)
        out[:, g0:hi] = flat[:, :hi - g0]
    return out
